# revision 58
# baseline (speedup 1.0000x reference)
"""Trainium2 Bass kernel for ActionExpertCrossBlock (dense transformer block
with GQA cross-attention + SwiGLU FFN), data-parallel over batch on 8 cores.

Contract: kernel(**inputs) takes the FULL fp32 inputs as produced by
setup_inputs() and returns the FULL [8, 512, 1024] fp32 output.

Per-core computation (batch element b):
  h   = rmsnorm(x) * ln1_w
  q   = rope((h @ Wq.T).reshape(L, 8, 256))
  k   = text_k @ Wk.T          (single KV head, shared by all 8 Q heads)
  v   = text_v @ Wv.T
  s_h = q_h @ k.T / 16         -> softmax over context
  ctx = attn @ v ; x2 = ctx @ Wo.T + x
  h2  = rmsnorm(x2) * ln2_w
  out = (silu(h2@Wg.T) * (h2@Wu.T)) @ Wd.T + x2

Precision: attention block (k/v-proj, q-proj, scores, softmax, attn@v,
o-proj) in fp8-e4m3 with DoubleRow matmuls (2 fp8 MACs per PE cell/cycle);
FFN stays bf16 (fp8 there blows the absmax error budget).  All quantization
scales are powers of two folded into existing constants.  Scores are computed
TRANSPOSED ([c, l]) so attn@v needs no on-chip transpose; the softmax
denominator uses a DoubleRow ones-matmul; exp = exp(s/16 - 3.5) stays < 240
(TRN e4m3 overflow).

Schedule (v2): the q-projection + RoPE of head h+1 are interleaved INTO head
h's attention stream (PE: 2 qproj half-groups; DVE: psum->bf16 copies + rope)
so there is no standalone q-proj phase with ACT idle; exp activations are
emitted 1024-wide (one per score PAIR, psum tile [P,2,L] spanning 2 banks) to
halve ACT instruction overhead -- ACT (8 exps/head ~9.5us) then just undercuts
PE (~10.3us/head).  The softmax reciprocal uses the ~5x faster
reciprocal_approx_fast (18 bits >> fp8 needs).  All bulk weight DMAs ride the
sync/gpsimd HWDGE rings so the Scalar(ACT) and Vector(DVE) queues carry no
descriptor work mid-kernel.  o-proj opens lt0/lt1 psum groups with heads 0-6
while head 7's recip/ctxT drain, then finishes with head 7.  PSUM map
(8 banks): PR 2x[P,2,L] (scores pairs / o-proj lt-groups / FFN gate+up
pairs), CX 2x[P,L] (k-proj / attn@v / norm2 transposes / down-proj), QP
1x[P,L] (qproj halves / norm1 transposes), DN 1x[P,L] (softmax denominator /
norm1 transposes).
"""
import sys

sys.path.insert(0, "/opt/trn_rl_repo")

import numpy as np
import ml_dtypes

import concourse.bass as bass
from concourse import bacc
import concourse.mybir as mybir
import concourse.tile as tile
from concourse.masks import make_identity
from concourse.bass_utils import run_bass_kernel_spmd

import os as _os
_DBG = _os.environ.get("ANT_DBG", "0") == "1"

P = 128
B, L, D = 8, 512, 1024
QH, HD = 8, 256
E = 256        # kv dim (1 head x 256)
LC = 2048      # context length
F = 4096       # ffn dim
O = QH * HD    # 2048
LT, DT, OT, CT, FTL = L // P, D // P, O // P, LC // P, F // P  # 4 8 16 16 32
f32, bf16, f8 = mybir.dt.float32, mybir.dt.bfloat16, mybir.dt.float8e4
DR = mybir.MatmulPerfMode.DoubleRow
EPS = float(np.finfo(np.float32).eps)
EXPF = mybir.ActivationFunctionType.Exp
SILU = mybir.ActivationFunctionType.Silu
SQRT = mybir.ActivationFunctionType.Sqrt
COPY = mybir.ActivationFunctionType.Copy
MUL = mybir.AluOpType.mult
SUB = mybir.AluOpType.subtract
ADD = mybir.AluOpType.add

# fp8 scale schedule (all powers of 2; see module docstring)
S_H = 16.0      # h = s_h * rmsnorm(x), fp8
S_WQ = 256.0    # Wq host-quant scale
S_Q = 16.0      # rope(q) fp8 scale; rope tables carry s_q/(s_h*s_wq) = 2^-8
S_K = 16.0      # k fp8 scale (copy from kv-proj psum)
S_V = 16.0      # v fp8 scale
S_WK = 32.0     # Wk/Wv host-quant scale (kv-proj runs fp8 DR)
S_CTX = 8.0     # ctx fp8 scale; ones-matrix = s_v/s_ctx = 2 folds it in
S_WO = 512.0    # Wo host-quant scale
C1 = S_CTX * S_WO          # 4096: x host-prescale == device x2/out scale
EXP_SCALE = 1.0 / (16.0 * S_Q * S_K)   # 2^-12
EXP_BIAS = -3.5
NPRE = 12       # wg/wu chunks prefetched during attention


def _rope_tables():
    # Match reference _rope numerics (fp32 ops) for d=256, l=512; tables are
    # pre-multiplied by s_q/(s_h*s_wq) so the DVE rope muls emit s_q*rope(q).
    d2 = HD // 2
    ts = (10000.0 ** (2.0 / HD * np.arange(d2, dtype=np.float32))).astype(np.float32)
    rad = (np.arange(L, dtype=np.float32)[None, :] / ts[:, None]).astype(np.float32)
    rs = np.float32(S_Q / (S_H * S_WQ))
    return (np.cos(rad) * rs).astype(ml_dtypes.bfloat16), \
        (np.sin(rad) * rs).astype(ml_dtypes.bfloat16)


def build_program():
    # All inputs are host-pre-arranged to [P(partition), chunk, ...] layouts so
    # every DMA moves KB-sized contiguous per-partition elements (no gathers).
    nc = bacc.Bacc()
    x_d = nc.dram_tensor("x", [P, LT, D], bf16, kind="ExternalInput")  # C1-scaled
    tkT_d = nc.dram_tensor("tkT", [P, 4, 2, 512], f8, kind="ExternalInput")  # x16
    tvT_d = nc.dram_tensor("tvT", [P, 4, 2, 512], f8, kind="ExternalInput")  # x16
    wqT_d = nc.dram_tensor("wqT", [P, 4, DT, 512], f8, kind="ExternalInput")
    wkT_d = nc.dram_tensor("wkT", [P, 2, E], f8, kind="ExternalInput")   # x32
    wvT_d = nc.dram_tensor("wvT", [P, 2, E], f8, kind="ExternalInput")   # x32
    woT_d = nc.dram_tensor("woT", [P, OT // 2, 2, 2, 512], f8, kind="ExternalInput")
    wgT_d = nc.dram_tensor("wgT", [P, FTL, DT, P], bf16, kind="ExternalInput")
    wuT_d = nc.dram_tensor("wuT", [P, FTL, DT, P], bf16, kind="ExternalInput")
    wdT_d = nc.dram_tensor("wdT", [P, FTL, D], bf16, kind="ExternalInput")
    out_d = nc.dram_tensor("out", [L, D], f32, kind="ExternalOutput")  # C1 * out

    cos_np, sin_np = _rope_tables()
    cosT_d = nc.inline_tensor(cos_np, "cosT")
    sinT_d = nc.inline_tensor(sin_np, "sinT")

    with tile.TileContext(nc) as tc:
        build_tile_kernel(
            tc, x_d, tkT_d, tvT_d, wqT_d, wkT_d, wvT_d, woT_d, wgT_d, wuT_d,
            wdT_d, cosT_d, sinT_d, out_d,
        )
    nc.compile()
    return nc


def _rmsnorm_lt(nc, pool, x_lt, dst_lt, sq_scale, eps_sb, tag, lt):
    """dst_lt = x_lt * rsqrt(ssum*sq_scale + eps_bias); sum-of-squares on ACT.

    sq_scale/eps_sb fold the h quantization scale and the host x prescale:
    dst = (s_out / (C * sqrt(mean((x/C)^2) + eps))) * x  for x = C*x_real,
    with sq_scale = 1/(D*s_out^2) and eps_bias = (C/s_out)^2 * eps.
    """
    sq = pool.tile([P, D], f32, tag=f"{tag}_sq", bufs=2, name=f"{tag}sq{lt}")
    ssum = pool.tile([P, 1], f32, tag=f"{tag}_sum", bufs=2, name=f"{tag}sm{lt}")
    nc.scalar.activation(sq, x_lt, mybir.ActivationFunctionType.Square,
                         accum_out=ssum)
    std = pool.tile([P, 1], f32, tag=f"{tag}_std", bufs=2, name=f"{tag}sd{lt}")
    nc.scalar.activation(std, ssum, SQRT, scale=sq_scale, bias=eps_sb)
    rstd = pool.tile([P, 1], f32, tag=f"{tag}_rstd", bufs=2, name=f"{tag}rs{lt}")
    nc.vector.reciprocal(rstd, std)
    nc.vector.tensor_scalar_mul(dst_lt[:, :D // 2], x_lt[:, :D // 2], rstd)
    nc.vector.tensor_scalar_mul(dst_lt[:, D // 2:], x_lt[:, D // 2:], rstd)


def build_tile_kernel(tc, x_d, tkT_d, tvT_d, wqT_d, wkT_d, wvT_d, woT_d,
                      wgT_d, wuT_d, wdT_d, cosT_d, sinT_d, out_d):
    nc = tc.nc

    consts = tc.alloc_tile_pool(name="consts", bufs=1)
    persist2 = tc.alloc_tile_pool(name="persist2", bufs=1)  # x2: lives to the end
    persistH = tc.alloc_tile_pool(name="persistH", bufs=1)  # h2T: E..G
    persist1 = tc.alloc_tile_pool(name="persist1", bufs=1)  # dead after o-proj

    ident = consts.tile([P, P], bf16)
    ones8 = consts.tile([P, 2, P], f8)   # value 2 = s_v/s_ctx folded into den
    eps1_sb = consts.tile([P, 1], f32)   # (C1/S_H)^2 * eps
    eps2_sb = consts.tile([P, 1], f32)   # C1^2 * eps (h2 unscaled)
    ebias_sb = consts.tile([P, 1], f32)  # exp bias (cancels in softmax)
    cosT = consts.tile([P, L], bf16)
    sinT = consts.tile([P, L], bf16)

    # ---- persistent activations (split into per-slice tiles so consumers
    # depend only on the slices they read, not on whole-tensor last-writes) ----
    qT_t = [persist1.tile([P, 2, L], f8, tag="qT", bufs=QH, name=f"qT{h}")
            for h in range(QH)]
    kT = persist1.tile([P, 2, LC], f8)
    v_sb = persist1.tile([P, CT, E], f8)
    ctxT_t = [persist1.tile([P, 2, L], f8, tag="ctxT", bufs=QH, name=f"ctxT{h}")
              for h in range(QH)]
    x_sb = persist1.tile([P, LT, D], bf16)
    x_t = [x_sb[:, lt, :] for lt in range(LT)]
    hT = persist1.tile([P, DT, L], f8)   # norm1(x) transposed, fp8 (S_H)
    x2_sb = persist2.tile([P, LT, D], f32)

    patt = tc.alloc_tile_pool(name="patt", bufs=1)   # attention-phase tiles
    pexp = tc.alloc_tile_pool(name="pexp", bufs=1)   # attention exp/recip tiles
    psum = tc.alloc_tile_pool(name="psum", bufs=2, space="PSUM")

    # ======== input DMAs: tiny/early loads spread over all HWDGE rings
    # (scalar only carries startup loads; mid-kernel descriptor work stays
    # on sync+gpsimd so the ACT queue is compute-only) ========
    wk_sb = patt.tile([P, 2, E], f8)
    nc.scalar.dma_start(wk_sb, wkT_d.ap())
    wv_sb = patt.tile([P, 2, E], f8)
    nc.scalar.dma_start(wv_sb, wvT_d.ap())

    # tk chunks alternate sync/scalar so the k-proj chunk stream is paced by
    # two rings; x follows on scalar (first needed ~12us)
    tk_c = []
    for cc in range(4):
        t = patt.tile([P, 2, 512], f8, tag="tk", bufs=4, name=f"tk{cc}")
        (nc.sync if cc % 2 == 0 else nc.scalar).dma_start(
            t, tkT_d.ap()[:, cc, :, :])
        tk_c.append(t)

    # x chunks split across sync+scalar so neither ring serializes the 1MB
    # behind the tk stream (norm1 lt=3 gates the whole q-proj/rope chain)
    for lt in range(LT):
        (nc.sync if lt % 2 == 0 else nc.scalar).dma_start(
            x_sb[:, lt, :], x_d.ap()[:, lt, :])

    tv_c = []
    for cc in range(4):
        t = patt.tile([P, 2, 512], f8, tag="tv", bufs=4, name=f"tv{cc}")
        nc.gpsimd.dma_start(t, tvT_d.ap()[:, cc, :, :])
        tv_c.append(t)
    nc.gpsimd.dma_start(cosT, cosT_d.ap())
    nc.gpsimd.dma_start(sinT, sinT_d.ap())
    wq_sb = patt.tile([P, 4, DT, 512], f8)
    # identity after the startup-critical gpsimd DMAs (needed ~13us)
    make_identity(nc, ident)
    wq_c = [wq_sb[:, h // 2, :, (h % 2) * 256:(h % 2) * 256 + 256]
            for h in range(QH)]

    nc.vector.memset(ones8, 2.0)
    nc.vector.memset(eps1_sb, (C1 / S_H) ** 2 * EPS)
    nc.vector.memset(eps2_sb, C1 * C1 * EPS)
    nc.vector.memset(ebias_sb, EXP_BIAS)

    # ====== k/v projections (fp8 DoubleRow) interleaved with norm1 and the
    # hT transposes -- the transposes fill PE while later chunks stream ======
    KSC = S_K / (16.0 * S_WK)   # psum = (16*tk)(32*wk) = 512*k
    VSC = S_V / (16.0 * S_WK)
    h_lts = {}

    def emit_norm1(lt):
        h_lt = patt.tile([P, D], bf16, tag="h_bf", bufs=4, name=f"hbf{lt}")
        _rmsnorm_lt(nc, patt, x_t[lt], h_lt, 1.0 / (D * S_H * S_H),
                    eps1_sb, "n1", lt)
        h_lts[lt] = h_lt

    def emit_trans1(lt, wide=False):
        # 4-way psum rotation once the kv matmuls (CX/PR users) are emitted
        tags = ("QP", "DN", "CX", "PR") if wide else ("QP", "DN")
        bufs = {"QP": 1, "DN": 1, "CX": 2, "PR": 2}
        for dt in range(DT):
            tag = tags[dt % len(tags)]
            tp = psum.tile([P, P], bf16, tag=tag, bufs=bufs[tag],
                           name=f"atp{lt}_{dt}")
            nc.tensor.transpose(tp, h_lts[lt][:, dt * P:(dt + 1) * P], ident)
            if dt % 2 == 0:
                nc.scalar.activation(hT[:, dt, lt * P:(lt + 1) * P], tp, COPY)
            else:
                nc.vector.tensor_copy(hT[:, dt, lt * P:(lt + 1) * P], tp)

    # chunk consumption follows DMA-arrival order: sync carries tk0/tk2 and
    # scalar tk1/tk3, so 0,2 land before 1,3
    cc_order = (0, 2, 1, 3)
    for idx in range(4):
        cc = cc_order[idx]
        c0 = cc * 512
        for et in range(2):
            psk = psum.tile([P, 512], f32, tag="CX", bufs=2,
                            name=f"psk{et}_{cc}")
            nc.tensor.matmul(psk, wk_sb[:, :, et * P:(et + 1) * P],
                             tk_c[cc], start=True, stop=True, perf_mode=DR)
            if (cc + et) % 2 == 0:
                nc.vector.tensor_scalar_mul(kT[:, et, c0:c0 + 512], psk, KSC)
            else:
                nc.scalar.activation(kT[:, et, c0:c0 + 512], psk, COPY,
                                     scale=KSC)
        for ct in range(4 * cc, 4 * cc + 4):
            psv = psum.tile([P, E], f32, tag="PR", bufs=2, name=f"psv{ct}")
            nc.tensor.matmul(
                psv, tv_c[cc][:, :, (ct % 4) * P:(ct % 4 + 1) * P],
                wv_sb, start=True, stop=True, perf_mode=DR)
            if ct % 2 == 0:
                nc.scalar.activation(v_sb[:, ct, :], psv, COPY, scale=VSC)
            else:
                nc.vector.tensor_scalar_mul(v_sb[:, ct, :], psv, VSC)
        emit_norm1(cc)
        if idx == 0:
            # preload the ACT exp table during prefill (a table switch costs
            # ~1.3us; don't pay it at head 0's first exp)
            dmex = patt.tile([P, 1], f8, name="dmex")
            nc.scalar.activation(dmex, eps1_sb, EXPF, scale=1.0, bias=ebias_sb)
            # wq chunks 1-3 (heads 2-7, not needed before ~55us) are gated
            # behind norm1(0)'s output so they stay out of the startup HBM
            # burst that the tk/tv/x chunk loads depend on
            nc.gpsimd.dma_start(wq_sb[:, 0], wqT_d.ap()[:, 0])
            for hp in range(1, 4):
                nc.vector.tensor_scalar_mul(wq_sb[0:1, hp, 0, 0:2],
                                            h_lts[0][0:1, 0:2], 0.0)
                nc.gpsimd.dma_start(wq_sb[:, hp], wqT_d.ap()[:, hp])
        if idx >= 1:
            emit_trans1(cc_order[idx - 1], wide=(idx == 3))
    emit_trans1(cc_order[3], wide=True)

    # ---- per-head q-proj + rope helpers (emitted inside head h-1's stream) --
    def qproj_half(g, half):
        psq = psum.tile([P, L], f32, tag="QP", bufs=1, name=f"psq{g}_{half}")
        for i in range(DT // 2):
            nc.tensor.matmul(
                psq, wq_c[g][:, 2 * i:2 * i + 2, half * P:(half + 1) * P],
                hT[:, 2 * i:2 * i + 2, :],
                start=(i == 0), stop=(i == DT // 2 - 1), perf_mode=DR)
        return psq

    def pqcopy(g, half, psq):
        t = patt.tile([P, L], bf16, tag="pqb", bufs=4, name=f"pqb{g}_{half}")
        nc.vector.tensor_copy(t, psq)
        return t

    def rope(g, pq0, pq1):
        # x1 = pq0, x2 = pq1 ([hd_j, l] layout; tables [j, l] carry the
        # s_q/(s_h*s_wq) rescale); outputs quantize to fp8.
        t_a = patt.tile([P, L], bf16, tag="rope_t", bufs=4, name=f"ta{g}")
        nc.vector.tensor_mul(t_a, pq0, cosT)
        t_b = patt.tile([P, L], bf16, tag="rope_t", bufs=4, name=f"tb{g}")
        nc.vector.tensor_mul(t_b, pq1, sinT)
        nc.vector.tensor_tensor(qT_t[g][:, 0, :], t_a, t_b, SUB)
        t_c = patt.tile([P, L], bf16, tag="rope_t", bufs=4, name=f"tc{g}")
        nc.vector.tensor_mul(t_c, pq1, cosT)
        t_d = patt.tile([P, L], bf16, tag="rope_t", bufs=4, name=f"td{g}")
        nc.vector.tensor_mul(t_d, pq0, sinT)
        nc.vector.tensor_tensor(qT_t[g][:, 1, :], t_c, t_d, ADD)

    # prefill: head 0's q-proj + rope, then head 1's first q-proj half (the
    # per-head loop emits half0 of head g at the END of head g-2's stream so
    # its psum copy lands on DVE before the boundary recip chain)
    psq0 = qproj_half(0, 0)
    pq0 = pqcopy(0, 0, psq0)
    psq1 = qproj_half(0, 1)
    pq1 = pqcopy(0, 1, psq1)
    rope(0, pq0, pq1)
    nxt = {}
    nxt[1] = qproj_half(1, 0)
    nxt[(1, "pq")] = pqcopy(1, 0, nxt[1])

    # ---- o-proj weight layout + FFN weight prefetch (issued once DVE reaches
    # the gate memsets, i.e. after head-0's rope -- keeps startup HBM clear) --
    pde = tc.alloc_tile_pool(name="ph_de", bufs=1)
    # wo layout [p, ot-pair, dc, parity, col]: each (pair, dc) moving slice is
    # a contiguous 1024B row pair -- strided DR moving operands run half-rate
    wo_sb = pde.tile([P, OT // 2, 2, 2, 512], f8)   # 16KB/part
    wg_pre = persistH.tile([P, NPRE, DT, P], bf16)
    wu_pre = persistH.tile([P, NPRE, DT, P], bf16)
    # WAW-gate the bulk prefetches on dummy writes that READ qT0 (produced by
    # head-0's rope): a bare memset has no deps and gets hoisted to t=0 by the
    # scheduler, and the 8MB prefetch then starves the startup loads of HBM
    nc.vector.tensor_scalar_mul(wo_sb[0:1, 0, 0, 0, 0:2], qT_t[0][0:1, 0, 0:2],
                                0.0)
    nc.vector.tensor_scalar_mul(wg_pre[0:1, 0, 0, 0:2], qT_t[0][0:1, 0, 0:2],
                                0.0)
    nc.vector.tensor_scalar_mul(wu_pre[0:1, 0, 0, 0:2], qT_t[0][0:1, 0, 0:2],
                                0.0)
    nc.sync.dma_start(wo_sb, woT_d.ap())
    nc.sync.dma_start(wg_pre, wgT_d.ap()[:, :NPRE])
    nc.gpsimd.dma_start(wu_pre, wuT_d.ap()[:, :NPRE])

    # ---- o-proj accumulator (dc-granular so head 7 can prefill lt0/dc0
    # into the QP bank while its own attention drains) ----
    pso_open = {}

    def oproj_acc(lt, dc, i0, i1, tag):
        if (lt, dc) not in pso_open:
            if tag == "PR":
                t = psum.tile([P, 2, 512], f32, tag="PR", bufs=2,
                              name=f"pso{lt}")
                pso_open[(lt, 0)] = t[:, 0, :]
                pso_open[(lt, 1)] = t[:, 1, :]
            else:
                pso_open[(lt, dc)] = psum.tile([P, 512], f32, tag=tag, bufs=1,
                                               name=f"pso{lt}_{dc}")
        pso = pso_open[(lt, dc)]
        for i in range(i0, i1):
            nc.tensor.matmul(
                pso, ctxT_t[i][:, :, lt * P:(lt + 1) * P],
                wo_sb[:, i, dc, :, :],
                start=(i == 0), stop=(i == OT // 2 - 1), perf_mode=DR)
        if i1 == OT // 2:
            nc.vector.tensor_tensor(
                x2_sb[:, lt, dc * 512:(dc + 1) * 512], pso,
                x_t[lt][:, dc * 512:(dc + 1) * 512], ADD)
            del pso_open[(lt, dc)]

    # ============ attention: per-head fused loop ============
    for h in range(QH):
        g = h + 1
        psc = [psum.tile([P, L], f32, tag="CX", bufs=2, name=f"psc{h}_{et}")
               for et in range(2)]
        psd = psum.tile([P, L], f32, tag="DN", bufs=1, name=f"psd{h}")
        exps = [None] * (CT // 2)

        def sp(p, h=h, exps=exps):
            # score pair p (ct = 2p, 2p+1) -> one 1024-wide fp8 exp on ACT
            pr = psum.tile([P, 2, L], f32, tag="PR", bufs=2, name=f"pr{h}_{p}")
            for j in range(2):
                ct = 2 * p + j
                nc.tensor.matmul(pr[:, j, :], kT[:, :, ct * P:(ct + 1) * P],
                                 qT_t[h], start=True, stop=True, perf_mode=DR)
            ex = pexp.tile([P, 2, L], f8, tag="exp", bufs=8, name=f"ex{h}_{p}")
            nc.scalar.activation(ex, pr, EXPF, scale=EXP_SCALE, bias=ebias_sb)
            exps[p] = ex

        def ctx(i, psc=psc, exps=exps):
            for et in range(2):
                nc.tensor.matmul(
                    psc[et], v_sb[:, 2 * i:2 * i + 2, et * P:(et + 1) * P],
                    exps[i], start=(i == 0), stop=(i == CT // 2 - 1),
                    perf_mode=DR)

        def den(i, psd=psd, exps=exps):
            nc.tensor.matmul(psd, ones8, exps[i],
                             start=(i == 0), stop=(i == CT // 2 - 1),
                             perf_mode=DR)

        # software pipeline: ctx(0) is deferred so its psum WAR (prev head's
        # ctxT muls draining on DVE) clears before PE arrives; head g's
        # q-proj half0 was emitted at the end of head h-1 (copy on DVE before
        # the boundary recip), half1+rope ride in this head's early slots
        sp(0)
        sp(1)
        if g < QH:
            psq1 = qproj_half(g, 1)
        sp(2)
        if g < QH:
            pq1 = pqcopy(g, 1, psq1)
        ctx(0)
        sp(3)
        if g < QH:
            rope(g, nxt[(g, "pq")], pq1)
        ctx(1)
        den(0)
        for p in range(4, CT // 2):
            sp(p)
            ctx(p - 2)
            den(p - 3)
        ctx(CT // 2 - 2)
        den(CT // 2 - 3)
        ctx(CT // 2 - 1)
        den(CT // 2 - 2)
        den(CT // 2 - 1)
        if g + 1 < QH:
            # next-next head's q-proj half0 + its DVE copy, ahead of the
            # boundary reciprocal so PE's half1 never waits on the copy
            nxt[g + 1] = qproj_half(g + 1, 0)
            nxt[(g + 1, "pq")] = pqcopy(g + 1, 0, nxt[g + 1])

        if _DBG and h == 0:
            dbg_ex = nc.dram_tensor("dbg_ex0", [P, CT // 2, 2, L], f8,
                                    kind="ExternalOutput")
            for p in range(CT // 2):
                nc.sync.dma_start(dbg_ex.ap()[:, p, :, :], exps[p])
        recip = pexp.tile([P, L], f32, tag="recip", bufs=2, name=f"rc{h}")
        nc.vector.reciprocal_approx_fast(recip, psd)
        for et in range(2):
            nc.vector.tensor_mul(ctxT_t[h][:, et, :], psc[et], recip)
        if _DBG:
            nc.sync.dma_start(
                nc.dram_tensor(f"dbg_rc{h}", [P, L], f32,
                               kind="ExternalOutput").ap(), recip)

    # ------- o-proj (fp8 DR) + residual, interleaved with norm2 ---------
    # lt0/dc0 was prefilled (heads 0..5) inside head 7's stream; lt1/lt2 use
    # the PR pair tiles, lt0/lt3 the QP+DN single banks
    h2T = persistH.tile([P, DT, L], bf16)

    def emit_norm2_rms(lt, h2_lt):
        _rmsnorm_lt(nc, pde, x2_sb[:, lt, :], h2_lt, 1.0 / D, eps2_sb,
                    "n2", lt)

    def emit_norm2_T(lt, h2_lt):
        for dt in range(DT):
            tp = psum.tile([P, P], bf16, tag="CX", bufs=2, name=f"ftp{lt}_{dt}")
            nc.tensor.transpose(tp, h2_lt[:, dt * P:(dt + 1) * P], ident)
            if dt % 2 == 0:
                nc.scalar.activation(h2T[:, dt, lt * P:(lt + 1) * P], tp, COPY)
            else:
                nc.vector.tensor_copy(h2T[:, dt, lt * P:(lt + 1) * P], tp)

    h2_t = [pde.tile([P, D], bf16, tag="h2bf", bufs=4, name=f"h2bf{lt}")
            for lt in range(LT)]
    # PR groups first: their psum slots have no dependence on head 7's
    # recip/ctxT drain (the QP/DN slots WAR-wait the psd read), and by the
    # time the head-7 stop-matmuls arrive ctxT7 is long ready
    QD = ("QP", "DN")
    for dc in range(2):
        oproj_acc(1, dc, 0, OT // 2, "PR")
    emit_norm2_rms(1, h2_t[1])
    for dc in range(2):
        oproj_acc(2, dc, 0, OT // 2, "PR")
    emit_norm2_T(1, h2_t[1])
    emit_norm2_rms(2, h2_t[2])
    for dc in range(2):
        oproj_acc(0, dc, 0, OT // 2, QD[dc])
    emit_norm2_T(2, h2_t[2])
    emit_norm2_rms(0, h2_t[0])
    for dc in range(2):
        oproj_acc(3, dc, 0, OT // 2, QD[dc])
    emit_norm2_T(0, h2_t[0])
    emit_norm2_rms(3, h2_t[3])
    emit_norm2_T(3, h2_t[3])

    if _DBG:
        nc.sync.dma_start(nc.dram_tensor("dbg_kT", [P, 2, LC], f8,
                                         kind="ExternalOutput").ap(), kT)
        nc.sync.dma_start(nc.dram_tensor("dbg_v", [P, CT, E], f8,
                                         kind="ExternalOutput").ap(), v_sb)
        nc.sync.dma_start(nc.dram_tensor("dbg_hT", [P, DT, L], f8,
                                         kind="ExternalOutput").ap(), hT)
        for h in range(QH):
            nc.sync.dma_start(nc.dram_tensor(f"dbg_qT{h}", [P, 2, L], f8,
                                             kind="ExternalOutput").ap(), qT_t[h])
            nc.sync.dma_start(nc.dram_tensor(f"dbg_ctxT{h}", [P, 2, L], f8,
                                             kind="ExternalOutput").ap(), ctxT_t[h])
        nc.sync.dma_start(nc.dram_tensor("dbg_x2", [P, LT, D], f32,
                                         kind="ExternalOutput").ap(), x2_sb)

    pde.release()
    pexp.release()
    patt.release()
    persist1.release()

    # ================= FFN (bf16) =================
    pfg = tc.alloc_tile_pool(name="ph_fg", bufs=1)
    fT = pfg.tile([P, FTL, L], bf16)          # 32KB/part

    wd_sb = pfg.tile([P, FTL, D], bf16)       # 64KB/part
    for ft in range(FTL):
        if ft < NPRE:
            wg_c = wg_pre[:, ft]
            wu_c = wu_pre[:, ft]
        else:
            wg_c = pfg.tile([P, DT, P], bf16, tag="wg", bufs=4, name=f"wg{ft}")
            nc.sync.dma_start(wg_c, wgT_d.ap()[:, ft])
            wu_c = pfg.tile([P, DT, P], bf16, tag="wu", bufs=4, name=f"wu{ft}")
            nc.gpsimd.dma_start(wu_c, wuT_d.ap()[:, ft])
        if ft % 4 == 2:
            # down-proj weights stream as 1MB chunks alternating across both
            # rings, interleaved with the g/u chunk stream
            i = ft // 4
            ring = nc.sync if i % 2 == 0 else nc.gpsimd
            ring.dma_start(wd_sb[:, 4 * i:4 * i + 4, :],
                           wdT_d.ap()[:, 4 * i:4 * i + 4, :])

        pg = psum.tile([P, 2, L], f32, tag="PR", bufs=2, name=f"pg{ft}")
        for dt in range(DT):
            nc.tensor.matmul(pg[:, 0, :], wg_c[:, dt, :], h2T[:, dt, :],
                             start=(dt == 0), stop=(dt == DT - 1))
        for dt in range(DT):
            nc.tensor.matmul(pg[:, 1, :], wu_c[:, dt, :], h2T[:, dt, :],
                             start=(dt == 0), stop=(dt == DT - 1))
        sl = pfg.tile([P, L], f32, tag="sl", bufs=2, name=f"sl{ft}")
        if _DBG:
            # CoreSim lacks Silu; emulate with sigmoid+mul for debugging
            nc.scalar.activation(sl, pg[:, 0, :],
                                 mybir.ActivationFunctionType.Sigmoid)
            sl2 = pfg.tile([P, L], f32, tag="sl2", bufs=2, name=f"sl2{ft}")
            nc.vector.tensor_mul(sl2, sl, pg[:, 0, :])
            sl = sl2
        else:
            nc.scalar.activation(sl, pg[:, 0, :], SILU)
        nc.vector.tensor_mul(fT[:, ft, :], sl, pg[:, 1, :])

    # down proj + residual + store (out carries C1; host divides); the final
    # chunk's adds alternate DVE/GpSimd and its stores use sync+scalar so no
    # single queue serializes the drain
    out_r = out_d.ap().rearrange("(lt p) d -> p lt d", p=P)
    rings = [nc.sync, nc.gpsimd, nc.scalar]
    for lt in range(LT):
        o_lt = pfg.tile([P, D], f32, tag="out", bufs=2, name=f"out{lt}")
        for dc in range(D // 512):
            # the very last chunk runs as two 256-col psum groups so its
            # add+store drain overlaps the second group's matmuls
            last = (lt == LT - 1 and dc == 1)
            ngr = 2 if last else 1
            wgr = 512 // ngr
            for gr in range(ngr):
                g0 = dc * 512 + gr * wgr
                psdn = psum.tile([P, wgr], f32, tag="CX", bufs=2,
                                 name=f"psdn{lt}_{dc}_{gr}")
                for ft in range(FTL):
                    nc.tensor.matmul(
                        psdn, fT[:, ft, lt * P:(lt + 1) * P],
                        wd_sb[:, ft, g0:g0 + wgr],
                        start=(ft == 0), stop=(ft == FTL - 1))
                nhf = 2
                wd_ = wgr // nhf
                for hf in range(nhf):
                    sl0 = g0 + hf * wd_
                    nc.vector.tensor_tensor(
                        o_lt[:, sl0:sl0 + wd_],
                        psdn[:, hf * wd_:(hf + 1) * wd_],
                        x2_sb[:, lt, sl0:sl0 + wd_], ADD)
                    ring = ([nc.sync, nc.scalar][(2 * gr + hf) % 2] if last
                            else rings[(2 * lt + dc * nhf + hf) % 3])
                    ring.dma_start(out_r[:, lt, sl0:sl0 + wd_],
                                   o_lt[:, sl0:sl0 + wd_])
    pfg.release()
    psum.release()
    persistH.release()
    persist2.release()
    consts.release()


def _to_bf16(a):
    return np.ascontiguousarray(a.astype(ml_dtypes.bfloat16))


def _to_f8(a, scale):
    y = np.asarray(a, np.float32) * np.float32(scale)
    np.clip(y, -240.0, 240.0, out=y)
    return np.ascontiguousarray(y.astype(ml_dtypes.float8_e4m3fn))


def prepare_core_inputs(x, text_k, text_v, ln1_w, ln2_w, Wq, Wk, Wv, Wo, Wg, Wu, Wd):
    """Host-side preprocessing: transpose weights, fold RMSNorm gammas,
    quantize (fp8 for Wq/Wk/Wv/Wo, bf16 elsewhere), prescale x by C1, and
    pre-arrange every tensor to the device [P, chunk, ...] layout so DMAs
    move KB-contiguous per-partition elements."""
    x = np.asarray(x, np.float32)

    def arr_pmaj(a):
        # [N*P, M] -> [P, N, M] (N chunk-major per partition)
        n = a.shape[0] // P
        return np.ascontiguousarray(
            a.reshape(n, P, a.shape[1]).transpose(1, 0, 2))

    def arr_kv(a):
        # [E, LC] -> [P, cc(4), ft(2), 512]: chunk-major contiguous per part
        return np.ascontiguousarray(
            a.reshape(2, P, 4, 512).transpose(1, 2, 0, 3))

    wq = _to_f8((np.asarray(Wq) * np.asarray(ln1_w)[None, :]).T, S_WQ)  # [D, O]
    # [D, O] -> [P, DT, O] -> chunks of 512 o-cols, hp-major: [P, 4, DT, 512]
    wq = wq.reshape(DT, P, 4, 512).transpose(1, 2, 0, 3)
    wo = _to_f8(np.asarray(Wo).T, S_WO)                 # [O, D]
    wo = wo.reshape(OT // 2, 2, P, 2, 512).transpose(2, 0, 3, 1, 4)
    wg = _to_bf16((np.asarray(Wg) * np.asarray(ln2_w)[None, :]).T)  # [D, F]
    wg = wg.reshape(DT, P, FTL, P).transpose(1, 2, 0, 3)            # [P,FTL,DT,P]
    # Wu carries C1 so the down-proj PSUM matches x2_sb's scale in the
    # final residual add (host divides the output by C1)
    wu = _to_bf16((np.asarray(Wu) * np.asarray(ln2_w)[None, :]).T
                  * np.float32(C1))
    wu = wu.reshape(DT, P, FTL, P).transpose(1, 2, 0, 3)
    shared = {
        "wqT": np.ascontiguousarray(wq),
        "wkT": arr_pmaj(_to_f8(np.asarray(Wk).T, S_WK)),
        "wvT": arr_pmaj(_to_f8(np.asarray(Wv).T, S_WK)),
        "woT": np.ascontiguousarray(wo),
        "wgT": np.ascontiguousarray(wg),
        "wuT": np.ascontiguousarray(wu),
        "wdT": arr_pmaj(_to_bf16(np.asarray(Wd).T)),   # [P, FTL, D]
    }
    in_maps = []
    for b in range(B):
        in_maps.append({
            "x": arr_pmaj(_to_bf16(np.asarray(x[b], np.float32)
                                   * np.float32(C1))),
            "tkT": arr_kv(_to_f8(np.asarray(text_k[b]).T, 16.0)),
            "tvT": arr_kv(_to_f8(np.asarray(text_v[b]).T, 16.0)),
            **shared,
        })
    return in_maps


_NC_CACHE = {}


def kernel(**inputs):
    if "nc" not in _NC_CACHE:
        _NC_CACHE["nc"] = build_program()
    nc = _NC_CACHE["nc"]
    in_maps = prepare_core_inputs(**inputs)
    res = run_bass_kernel_spmd(nc, in_maps, core_ids=list(range(B)))
    inv = np.float32(1.0 / C1)
    return np.stack([r["out"] * inv for r in res.results], axis=0)


if __name__ == "__main__":
    # smoke build
    nc = build_program()
    print("program built ok")


# revision 59
# speedup vs baseline: 1.0040x; 1.0040x over previous
"""Trainium2 Bass kernel for ActionExpertCrossBlock (dense transformer block
with GQA cross-attention + SwiGLU FFN), data-parallel over batch on 8 cores.

Contract: kernel(**inputs) takes the FULL fp32 inputs as produced by
setup_inputs() and returns the FULL [8, 512, 1024] fp32 output.

Per-core computation (batch element b):
  h   = rmsnorm(x) * ln1_w
  q   = rope((h @ Wq.T).reshape(L, 8, 256))
  k   = text_k @ Wk.T          (single KV head, shared by all 8 Q heads)
  v   = text_v @ Wv.T
  s_h = q_h @ k.T / 16         -> softmax over context
  ctx = attn @ v ; x2 = ctx @ Wo.T + x
  h2  = rmsnorm(x2) * ln2_w
  out = (silu(h2@Wg.T) * (h2@Wu.T)) @ Wd.T + x2

Precision: attention block (k/v-proj, q-proj, scores, softmax, attn@v,
o-proj) in fp8-e4m3 with DoubleRow matmuls (2 fp8 MACs per PE cell/cycle);
FFN stays bf16 (fp8 there blows the absmax error budget).  All quantization
scales are powers of two folded into existing constants.  Scores are computed
TRANSPOSED ([c, l]) so attn@v needs no on-chip transpose; the softmax
denominator uses a DoubleRow ones-matmul; exp = exp(s/16 - 3.5) stays < 240
(TRN e4m3 overflow).

Schedule (v2): the q-projection + RoPE of head h+1 are interleaved INTO head
h's attention stream (PE: 2 qproj half-groups; DVE: psum->bf16 copies + rope)
so there is no standalone q-proj phase with ACT idle; exp activations are
emitted 1024-wide (one per score PAIR, psum tile [P,2,L] spanning 2 banks) to
halve ACT instruction overhead -- ACT (8 exps/head ~9.5us) then just undercuts
PE (~10.3us/head).  The softmax reciprocal uses the ~5x faster
reciprocal_approx_fast (18 bits >> fp8 needs).  All bulk weight DMAs ride the
sync/gpsimd HWDGE rings so the Scalar(ACT) and Vector(DVE) queues carry no
descriptor work mid-kernel.  o-proj opens lt0/lt1 psum groups with heads 0-6
while head 7's recip/ctxT drain, then finishes with head 7.  PSUM map
(8 banks): PR 2x[P,2,L] (scores pairs / o-proj lt-groups / FFN gate+up
pairs), CX 2x[P,L] (k-proj / attn@v / norm2 transposes / down-proj), QP
1x[P,L] (qproj halves / norm1 transposes), DN 1x[P,L] (softmax denominator /
norm1 transposes).
"""
import sys

sys.path.insert(0, "/opt/trn_rl_repo")

import numpy as np
import ml_dtypes

import concourse.bass as bass
from concourse import bacc
import concourse.mybir as mybir
import concourse.tile as tile
from concourse.masks import make_identity
from concourse.bass_utils import run_bass_kernel_spmd

import os as _os
_DBG = _os.environ.get("ANT_DBG", "0") == "1"

P = 128
B, L, D = 8, 512, 1024
QH, HD = 8, 256
E = 256        # kv dim (1 head x 256)
LC = 2048      # context length
F = 4096       # ffn dim
O = QH * HD    # 2048
LT, DT, OT, CT, FTL = L // P, D // P, O // P, LC // P, F // P  # 4 8 16 16 32
f32, bf16, f8 = mybir.dt.float32, mybir.dt.bfloat16, mybir.dt.float8e4
DR = mybir.MatmulPerfMode.DoubleRow
EPS = float(np.finfo(np.float32).eps)
EXPF = mybir.ActivationFunctionType.Exp
SILU = mybir.ActivationFunctionType.Silu
SQRT = mybir.ActivationFunctionType.Sqrt
COPY = mybir.ActivationFunctionType.Copy
MUL = mybir.AluOpType.mult
SUB = mybir.AluOpType.subtract
ADD = mybir.AluOpType.add

# fp8 scale schedule (all powers of 2; see module docstring)
S_H = 16.0      # h = s_h * rmsnorm(x), fp8
S_WQ = 256.0    # Wq host-quant scale
S_Q = 16.0      # rope(q) fp8 scale; rope tables carry s_q/(s_h*s_wq) = 2^-8
S_K = 16.0      # k fp8 scale (copy from kv-proj psum)
S_V = 16.0      # v fp8 scale
S_WK = 32.0     # Wk/Wv host-quant scale (kv-proj runs fp8 DR)
S_CTX = 8.0     # ctx fp8 scale; ones-matrix = s_v/s_ctx = 2 folds it in
S_WO = 512.0    # Wo host-quant scale
C1 = S_CTX * S_WO          # 4096: x host-prescale == device x2/out scale
EXP_SCALE = 1.0 / (16.0 * S_Q * S_K)   # 2^-12
EXP_BIAS = -3.5
NPRE = 12       # wg/wu chunks prefetched during attention


def _rope_tables():
    # Match reference _rope numerics (fp32 ops) for d=256, l=512; tables are
    # pre-multiplied by s_q/(s_h*s_wq) so the DVE rope muls emit s_q*rope(q).
    d2 = HD // 2
    ts = (10000.0 ** (2.0 / HD * np.arange(d2, dtype=np.float32))).astype(np.float32)
    rad = (np.arange(L, dtype=np.float32)[None, :] / ts[:, None]).astype(np.float32)
    rs = np.float32(S_Q / (S_H * S_WQ))
    return (np.cos(rad) * rs).astype(ml_dtypes.bfloat16), \
        (np.sin(rad) * rs).astype(ml_dtypes.bfloat16)


def build_program():
    # All inputs are host-pre-arranged to [P(partition), chunk, ...] layouts so
    # every DMA moves KB-sized contiguous per-partition elements (no gathers).
    nc = bacc.Bacc()
    x_d = nc.dram_tensor("x", [P, LT, D], bf16, kind="ExternalInput")  # C1-scaled
    tkT_d = nc.dram_tensor("tkT", [P, 4, 2, 512], f8, kind="ExternalInput")  # x16
    tvT_d = nc.dram_tensor("tvT", [P, 4, 2, 512], f8, kind="ExternalInput")  # x16
    wqT_d = nc.dram_tensor("wqT", [P, 4, DT, 512], f8, kind="ExternalInput")
    wkT_d = nc.dram_tensor("wkT", [P, 2, E], f8, kind="ExternalInput")   # x32
    wvT_d = nc.dram_tensor("wvT", [P, 2, E], f8, kind="ExternalInput")   # x32
    woT_d = nc.dram_tensor("woT", [P, OT // 2, 2, 2, 512], f8, kind="ExternalInput")
    wgT_d = nc.dram_tensor("wgT", [P, FTL, DT, P], bf16, kind="ExternalInput")
    wuT_d = nc.dram_tensor("wuT", [P, FTL, DT, P], bf16, kind="ExternalInput")
    wdT_d = nc.dram_tensor("wdT", [P, FTL, D], bf16, kind="ExternalInput")
    out_d = nc.dram_tensor("out", [L, D], f32, kind="ExternalOutput")  # C1 * out

    cos_np, sin_np = _rope_tables()
    cosT_d = nc.inline_tensor(cos_np, "cosT")
    sinT_d = nc.inline_tensor(sin_np, "sinT")

    with tile.TileContext(nc) as tc:
        build_tile_kernel(
            tc, x_d, tkT_d, tvT_d, wqT_d, wkT_d, wvT_d, woT_d, wgT_d, wuT_d,
            wdT_d, cosT_d, sinT_d, out_d,
        )
    nc.compile()
    return nc


def _rmsnorm_lt(nc, pool, x_lt, dst_lt, sq_scale, eps_sb, tag, lt):
    """dst_lt = x_lt * rsqrt(ssum*sq_scale + eps_bias); sum-of-squares on ACT.

    sq_scale/eps_sb fold the h quantization scale and the host x prescale:
    dst = (s_out / (C * sqrt(mean((x/C)^2) + eps))) * x  for x = C*x_real,
    with sq_scale = 1/(D*s_out^2) and eps_bias = (C/s_out)^2 * eps.
    """
    sq = pool.tile([P, D], f32, tag=f"{tag}_sq", bufs=2, name=f"{tag}sq{lt}")
    ssum = pool.tile([P, 1], f32, tag=f"{tag}_sum", bufs=2, name=f"{tag}sm{lt}")
    nc.scalar.activation(sq, x_lt, mybir.ActivationFunctionType.Square,
                         accum_out=ssum)
    std = pool.tile([P, 1], f32, tag=f"{tag}_std", bufs=2, name=f"{tag}sd{lt}")
    nc.scalar.activation(std, ssum, SQRT, scale=sq_scale, bias=eps_sb)
    rstd = pool.tile([P, 1], f32, tag=f"{tag}_rstd", bufs=2, name=f"{tag}rs{lt}")
    nc.vector.reciprocal(rstd, std)
    nc.vector.tensor_scalar_mul(dst_lt[:, :D // 2], x_lt[:, :D // 2], rstd)
    nc.vector.tensor_scalar_mul(dst_lt[:, D // 2:], x_lt[:, D // 2:], rstd)


def build_tile_kernel(tc, x_d, tkT_d, tvT_d, wqT_d, wkT_d, wvT_d, woT_d,
                      wgT_d, wuT_d, wdT_d, cosT_d, sinT_d, out_d):
    nc = tc.nc

    consts = tc.alloc_tile_pool(name="consts", bufs=1)
    persist2 = tc.alloc_tile_pool(name="persist2", bufs=1)  # x2: lives to the end
    persistH = tc.alloc_tile_pool(name="persistH", bufs=1)  # h2T: E..G
    persist1 = tc.alloc_tile_pool(name="persist1", bufs=1)  # dead after o-proj

    ident = consts.tile([P, P], bf16)
    ones8 = consts.tile([P, 2, P], f8)   # value 2 = s_v/s_ctx folded into den
    eps1_sb = consts.tile([P, 1], f32)   # (C1/S_H)^2 * eps
    eps2_sb = consts.tile([P, 1], f32)   # C1^2 * eps (h2 unscaled)
    ebias_sb = consts.tile([P, 1], f32)  # exp bias (cancels in softmax)
    cosT = consts.tile([P, L], bf16)
    sinT = consts.tile([P, L], bf16)

    # ---- persistent activations (split into per-slice tiles so consumers
    # depend only on the slices they read, not on whole-tensor last-writes) ----
    qT_t = [persist1.tile([P, 2, L], f8, tag="qT", bufs=QH, name=f"qT{h}")
            for h in range(QH)]
    kT = persist1.tile([P, 2, LC], f8)
    v_sb = persist1.tile([P, CT, E], f8)
    ctxT_t = [persist1.tile([P, 2, L], f8, tag="ctxT", bufs=QH, name=f"ctxT{h}")
              for h in range(QH)]
    x_sb = persist1.tile([P, LT, D], bf16)
    x_t = [x_sb[:, lt, :] for lt in range(LT)]
    hT = persist1.tile([P, DT, L], f8)   # norm1(x) transposed, fp8 (S_H)
    x2_sb = persist2.tile([P, LT, D], f32)

    patt = tc.alloc_tile_pool(name="patt", bufs=1)   # attention-phase tiles
    pexp = tc.alloc_tile_pool(name="pexp", bufs=1)   # attention exp/recip tiles
    psum = tc.alloc_tile_pool(name="psum", bufs=2, space="PSUM")

    # ======== input DMAs: tiny/early loads spread over all HWDGE rings
    # (scalar only carries startup loads; mid-kernel descriptor work stays
    # on sync+gpsimd so the ACT queue is compute-only) ========
    wk_sb = patt.tile([P, 2, E], f8)
    nc.scalar.dma_start(wk_sb, wkT_d.ap())
    wv_sb = patt.tile([P, 2, E], f8)
    nc.scalar.dma_start(wv_sb, wvT_d.ap())

    # tk chunks alternate sync/scalar so the k-proj chunk stream is paced by
    # two rings; x follows on scalar (first needed ~12us)
    tk_c = []
    for cc in range(4):
        t = patt.tile([P, 2, 512], f8, tag="tk", bufs=4, name=f"tk{cc}")
        (nc.sync if cc % 2 == 0 else nc.scalar).dma_start(
            t, tkT_d.ap()[:, cc, :, :])
        tk_c.append(t)

    # x chunks split across sync+scalar so neither ring serializes the 1MB
    # behind the tk stream (norm1 lt=3 gates the whole q-proj/rope chain)
    for lt in range(LT):
        (nc.sync if lt % 2 == 0 else nc.scalar).dma_start(
            x_sb[:, lt, :], x_d.ap()[:, lt, :])

    tv_c = []
    for cc in range(4):
        t = patt.tile([P, 2, 512], f8, tag="tv", bufs=4, name=f"tv{cc}")
        nc.gpsimd.dma_start(t, tvT_d.ap()[:, cc, :, :])
        tv_c.append(t)
    nc.gpsimd.dma_start(cosT, cosT_d.ap())
    nc.gpsimd.dma_start(sinT, sinT_d.ap())
    wq_sb = patt.tile([P, 4, DT, 512], f8)
    # identity after the startup-critical gpsimd DMAs (needed ~13us)
    make_identity(nc, ident)
    wq_c = [wq_sb[:, h // 2, :, (h % 2) * 256:(h % 2) * 256 + 256]
            for h in range(QH)]

    nc.vector.memset(ones8, 2.0)
    nc.vector.memset(eps1_sb, (C1 / S_H) ** 2 * EPS)
    nc.vector.memset(eps2_sb, C1 * C1 * EPS)
    nc.vector.memset(ebias_sb, EXP_BIAS)

    # ====== k/v projections (fp8 DoubleRow) interleaved with norm1 and the
    # hT transposes -- the transposes fill PE while later chunks stream ======
    KSC = S_K / (16.0 * S_WK)   # psum = (16*tk)(32*wk) = 512*k
    VSC = S_V / (16.0 * S_WK)
    h_lts = []

    def emit_norm1(lt):
        h_lt = patt.tile([P, D], bf16, tag="h_bf", bufs=4, name=f"hbf{lt}")
        _rmsnorm_lt(nc, patt, x_t[lt], h_lt, 1.0 / (D * S_H * S_H),
                    eps1_sb, "n1", lt)
        h_lts.append(h_lt)

    def emit_trans1(lt, wide=False):
        # 4-way psum rotation once the kv matmuls (CX/PR users) are emitted
        tags = ("QP", "DN", "CX", "PR") if wide else ("QP", "DN")
        bufs = {"QP": 1, "DN": 1, "CX": 2, "PR": 2}
        for dt in range(DT):
            tag = tags[dt % len(tags)]
            tp = psum.tile([P, P], bf16, tag=tag, bufs=bufs[tag],
                           name=f"atp{lt}_{dt}")
            nc.tensor.transpose(tp, h_lts[lt][:, dt * P:(dt + 1) * P], ident)
            if dt % 2 == 0:
                nc.scalar.activation(hT[:, dt, lt * P:(lt + 1) * P], tp, COPY)
            else:
                nc.vector.tensor_copy(hT[:, dt, lt * P:(lt + 1) * P], tp)

    for cc in range(4):
        c0 = cc * 512
        for et in range(2):
            psk = psum.tile([P, 512], f32, tag="CX", bufs=2,
                            name=f"psk{et}_{cc}")
            nc.tensor.matmul(psk, wk_sb[:, :, et * P:(et + 1) * P],
                             tk_c[cc], start=True, stop=True, perf_mode=DR)
            if (cc + et) % 2 == 0:
                nc.vector.tensor_scalar_mul(kT[:, et, c0:c0 + 512], psk, KSC)
            else:
                nc.scalar.activation(kT[:, et, c0:c0 + 512], psk, COPY,
                                     scale=KSC)
        for ct in range(4 * cc, 4 * cc + 4):
            psv = psum.tile([P, E], f32, tag="PR", bufs=2, name=f"psv{ct}")
            nc.tensor.matmul(
                psv, tv_c[cc][:, :, (ct % 4) * P:(ct % 4 + 1) * P],
                wv_sb, start=True, stop=True, perf_mode=DR)
            if ct % 2 == 0:
                nc.scalar.activation(v_sb[:, ct, :], psv, COPY, scale=VSC)
            else:
                nc.vector.tensor_scalar_mul(v_sb[:, ct, :], psv, VSC)
        emit_norm1(cc)
        if cc == 0:
            # preload the ACT exp table during prefill (a table switch costs
            # ~1.3us; don't pay it at head 0's first exp)
            dmex = patt.tile([P, 1], f8, name="dmex")
            nc.scalar.activation(dmex, eps1_sb, EXPF, scale=1.0, bias=ebias_sb)
            # wq chunks 1-3 (heads 2-7, not needed before ~55us) are gated
            # behind norm1(0)'s output so they stay out of the startup HBM
            # burst that the tk/tv/x chunk loads depend on
            nc.gpsimd.dma_start(wq_sb[:, 0], wqT_d.ap()[:, 0])
            for hp in range(1, 4):
                nc.vector.tensor_scalar_mul(wq_sb[0:1, hp, 0, 0:2],
                                            h_lts[0][0:1, 0:2], 0.0)
                nc.gpsimd.dma_start(wq_sb[:, hp], wqT_d.ap()[:, hp])
        if cc >= 1:
            emit_trans1(cc - 1, wide=(cc == 3))
    emit_trans1(LT - 1, wide=True)

    # ---- per-head q-proj + rope helpers (emitted inside head h-1's stream) --
    def qproj_half(g, half):
        psq = psum.tile([P, L], f32, tag="QP", bufs=1, name=f"psq{g}_{half}")
        for i in range(DT // 2):
            nc.tensor.matmul(
                psq, wq_c[g][:, 2 * i:2 * i + 2, half * P:(half + 1) * P],
                hT[:, 2 * i:2 * i + 2, :],
                start=(i == 0), stop=(i == DT // 2 - 1), perf_mode=DR)
        return psq

    def pqcopy(g, half, psq):
        t = patt.tile([P, L], bf16, tag="pqb", bufs=4, name=f"pqb{g}_{half}")
        nc.vector.tensor_copy(t, psq)
        return t

    def rope(g, pq0, pq1):
        # x1 = pq0, x2 = pq1 ([hd_j, l] layout; tables [j, l] carry the
        # s_q/(s_h*s_wq) rescale); outputs quantize to fp8.
        t_a = patt.tile([P, L], bf16, tag="rope_t", bufs=4, name=f"ta{g}")
        nc.vector.tensor_mul(t_a, pq0, cosT)
        t_b = patt.tile([P, L], bf16, tag="rope_t", bufs=4, name=f"tb{g}")
        nc.vector.tensor_mul(t_b, pq1, sinT)
        nc.vector.tensor_tensor(qT_t[g][:, 0, :], t_a, t_b, SUB)
        t_c = patt.tile([P, L], bf16, tag="rope_t", bufs=4, name=f"tc{g}")
        nc.vector.tensor_mul(t_c, pq1, cosT)
        t_d = patt.tile([P, L], bf16, tag="rope_t", bufs=4, name=f"td{g}")
        nc.vector.tensor_mul(t_d, pq0, sinT)
        nc.vector.tensor_tensor(qT_t[g][:, 1, :], t_c, t_d, ADD)

    # prefill: head 0's q-proj + rope, then head 1's first q-proj half (the
    # per-head loop emits half0 of head g at the END of head g-2's stream so
    # its psum copy lands on DVE before the boundary recip chain)
    psq0 = qproj_half(0, 0)
    pq0 = pqcopy(0, 0, psq0)
    psq1 = qproj_half(0, 1)
    pq1 = pqcopy(0, 1, psq1)
    rope(0, pq0, pq1)
    nxt = {}
    nxt[1] = qproj_half(1, 0)
    nxt[(1, "pq")] = pqcopy(1, 0, nxt[1])

    # ---- o-proj weight layout + FFN weight prefetch (issued once DVE reaches
    # the gate memsets, i.e. after head-0's rope -- keeps startup HBM clear) --
    pde = tc.alloc_tile_pool(name="ph_de", bufs=1)
    # wo layout [p, ot-pair, dc, parity, col]: each (pair, dc) moving slice is
    # a contiguous 1024B row pair -- strided DR moving operands run half-rate
    wo_sb = pde.tile([P, OT // 2, 2, 2, 512], f8)   # 16KB/part
    wg_pre = persistH.tile([P, NPRE, DT, P], bf16)
    wu_pre = persistH.tile([P, NPRE, DT, P], bf16)
    # WAW-gate the bulk prefetches on dummy writes that READ qT0 (produced by
    # head-0's rope): a bare memset has no deps and gets hoisted to t=0 by the
    # scheduler, and the 8MB prefetch then starves the startup loads of HBM
    nc.vector.tensor_scalar_mul(wo_sb[0:1, 0, 0, 0, 0:2], qT_t[0][0:1, 0, 0:2],
                                0.0)
    nc.vector.tensor_scalar_mul(wg_pre[0:1, 0, 0, 0:2], qT_t[0][0:1, 0, 0:2],
                                0.0)
    nc.vector.tensor_scalar_mul(wu_pre[0:1, 0, 0, 0:2], qT_t[0][0:1, 0, 0:2],
                                0.0)
    nc.sync.dma_start(wo_sb, woT_d.ap())
    nc.sync.dma_start(wg_pre, wgT_d.ap()[:, :NPRE])
    nc.gpsimd.dma_start(wu_pre, wuT_d.ap()[:, :NPRE])

    # ---- o-proj accumulator (dc-granular so head 7 can prefill lt0/dc0
    # into the QP bank while its own attention drains) ----
    pso_open = {}

    def oproj_acc(lt, dc, i0, i1, tag):
        if (lt, dc) not in pso_open:
            if tag == "PR":
                t = psum.tile([P, 2, 512], f32, tag="PR", bufs=2,
                              name=f"pso{lt}")
                pso_open[(lt, 0)] = t[:, 0, :]
                pso_open[(lt, 1)] = t[:, 1, :]
            else:
                pso_open[(lt, dc)] = psum.tile([P, 512], f32, tag=tag, bufs=1,
                                               name=f"pso{lt}_{dc}")
        pso = pso_open[(lt, dc)]
        for i in range(i0, i1):
            nc.tensor.matmul(
                pso, ctxT_t[i][:, :, lt * P:(lt + 1) * P],
                wo_sb[:, i, dc, :, :],
                start=(i == 0), stop=(i == OT // 2 - 1), perf_mode=DR)
        if i1 == OT // 2:
            nc.vector.tensor_tensor(
                x2_sb[:, lt, dc * 512:(dc + 1) * 512], pso,
                x_t[lt][:, dc * 512:(dc + 1) * 512], ADD)
            del pso_open[(lt, dc)]

    # ============ attention: per-head fused loop ============
    for h in range(QH):
        g = h + 1
        psc = [psum.tile([P, L], f32, tag="CX", bufs=2, name=f"psc{h}_{et}")
               for et in range(2)]
        psd = psum.tile([P, L], f32, tag="DN", bufs=1, name=f"psd{h}")
        exps = [None] * (CT // 2)

        def sp(p, h=h, exps=exps):
            # score pair p (ct = 2p, 2p+1) -> one 1024-wide fp8 exp on ACT
            pr = psum.tile([P, 2, L], f32, tag="PR", bufs=2, name=f"pr{h}_{p}")
            for j in range(2):
                ct = 2 * p + j
                nc.tensor.matmul(pr[:, j, :], kT[:, :, ct * P:(ct + 1) * P],
                                 qT_t[h], start=True, stop=True, perf_mode=DR)
            ex = pexp.tile([P, 2, L], f8, tag="exp", bufs=8, name=f"ex{h}_{p}")
            nc.scalar.activation(ex, pr, EXPF, scale=EXP_SCALE, bias=ebias_sb)
            exps[p] = ex

        def ctx(i, psc=psc, exps=exps):
            for et in range(2):
                nc.tensor.matmul(
                    psc[et], v_sb[:, 2 * i:2 * i + 2, et * P:(et + 1) * P],
                    exps[i], start=(i == 0), stop=(i == CT // 2 - 1),
                    perf_mode=DR)

        def den(i, psd=psd, exps=exps):
            nc.tensor.matmul(psd, ones8, exps[i],
                             start=(i == 0), stop=(i == CT // 2 - 1),
                             perf_mode=DR)

        # software pipeline: ctx(0) is deferred so its psum WAR (prev head's
        # ctxT muls draining on DVE) clears before PE arrives; head g's
        # q-proj half0 was emitted at the end of head h-1 (copy on DVE before
        # the boundary recip), half1+rope ride in this head's early slots
        sp(0)
        sp(1)
        if g < QH:
            psq1 = qproj_half(g, 1)
        sp(2)
        if g < QH:
            pq1 = pqcopy(g, 1, psq1)
        ctx(0)
        sp(3)
        if g < QH:
            rope(g, nxt[(g, "pq")], pq1)
        ctx(1)
        den(0)
        for p in range(4, CT // 2):
            sp(p)
            ctx(p - 2)
            den(p - 3)
        ctx(CT // 2 - 2)
        den(CT // 2 - 3)
        ctx(CT // 2 - 1)
        den(CT // 2 - 2)
        den(CT // 2 - 1)
        if g + 1 < QH:
            # next-next head's q-proj half0 + its DVE copy, ahead of the
            # boundary reciprocal so PE's half1 never waits on the copy
            nxt[g + 1] = qproj_half(g + 1, 0)
            nxt[(g + 1, "pq")] = pqcopy(g + 1, 0, nxt[g + 1])

        if _DBG and h == 0:
            dbg_ex = nc.dram_tensor("dbg_ex0", [P, CT // 2, 2, L], f8,
                                    kind="ExternalOutput")
            for p in range(CT // 2):
                nc.sync.dma_start(dbg_ex.ap()[:, p, :, :], exps[p])
        recip = pexp.tile([P, L], f32, tag="recip", bufs=2, name=f"rc{h}")
        nc.vector.reciprocal_approx_fast(recip, psd)
        for et in range(2):
            nc.vector.tensor_mul(ctxT_t[h][:, et, :], psc[et], recip)
        if _DBG:
            nc.sync.dma_start(
                nc.dram_tensor(f"dbg_rc{h}", [P, L], f32,
                               kind="ExternalOutput").ap(), recip)

    # ------- o-proj (fp8 DR) + residual, interleaved with norm2 ---------
    # lt0/dc0 was prefilled (heads 0..5) inside head 7's stream; lt1/lt2 use
    # the PR pair tiles, lt0/lt3 the QP+DN single banks
    h2T = persistH.tile([P, DT, L], bf16)

    def emit_norm2_rms(lt, h2_lt):
        _rmsnorm_lt(nc, pde, x2_sb[:, lt, :], h2_lt, 1.0 / D, eps2_sb,
                    "n2", lt)

    def emit_norm2_T(lt, h2_lt):
        for dt in range(DT):
            tp = psum.tile([P, P], bf16, tag="CX", bufs=2, name=f"ftp{lt}_{dt}")
            nc.tensor.transpose(tp, h2_lt[:, dt * P:(dt + 1) * P], ident)
            if dt % 2 == 0:
                nc.scalar.activation(h2T[:, dt, lt * P:(lt + 1) * P], tp, COPY)
            else:
                nc.vector.tensor_copy(h2T[:, dt, lt * P:(lt + 1) * P], tp)

    h2_t = [pde.tile([P, D], bf16, tag="h2bf", bufs=4, name=f"h2bf{lt}")
            for lt in range(LT)]
    # PR groups first: their psum slots have no dependence on head 7's
    # recip/ctxT drain (the QP/DN slots WAR-wait the psd read), and by the
    # time the head-7 stop-matmuls arrive ctxT7 is long ready
    QD = ("QP", "DN")
    for dc in range(2):
        oproj_acc(1, dc, 0, OT // 2, "PR")
    emit_norm2_rms(1, h2_t[1])
    for dc in range(2):
        oproj_acc(2, dc, 0, OT // 2, "PR")
    emit_norm2_T(1, h2_t[1])
    emit_norm2_rms(2, h2_t[2])
    for dc in range(2):
        oproj_acc(0, dc, 0, OT // 2, QD[dc])
    emit_norm2_T(2, h2_t[2])
    emit_norm2_rms(0, h2_t[0])
    for dc in range(2):
        oproj_acc(3, dc, 0, OT // 2, QD[dc])
    emit_norm2_T(0, h2_t[0])
    emit_norm2_rms(3, h2_t[3])
    emit_norm2_T(3, h2_t[3])

    if _DBG:
        nc.sync.dma_start(nc.dram_tensor("dbg_kT", [P, 2, LC], f8,
                                         kind="ExternalOutput").ap(), kT)
        nc.sync.dma_start(nc.dram_tensor("dbg_v", [P, CT, E], f8,
                                         kind="ExternalOutput").ap(), v_sb)
        nc.sync.dma_start(nc.dram_tensor("dbg_hT", [P, DT, L], f8,
                                         kind="ExternalOutput").ap(), hT)
        for h in range(QH):
            nc.sync.dma_start(nc.dram_tensor(f"dbg_qT{h}", [P, 2, L], f8,
                                             kind="ExternalOutput").ap(), qT_t[h])
            nc.sync.dma_start(nc.dram_tensor(f"dbg_ctxT{h}", [P, 2, L], f8,
                                             kind="ExternalOutput").ap(), ctxT_t[h])
        nc.sync.dma_start(nc.dram_tensor("dbg_x2", [P, LT, D], f32,
                                         kind="ExternalOutput").ap(), x2_sb)

    pde.release()
    pexp.release()
    patt.release()
    persist1.release()

    # ================= FFN (bf16) =================
    pfg = tc.alloc_tile_pool(name="ph_fg", bufs=1)
    fT = pfg.tile([P, FTL, L], bf16)          # 32KB/part

    wd_sb = pfg.tile([P, FTL, D], bf16)       # 64KB/part
    for ft in range(FTL):
        if ft < NPRE:
            wg_c = wg_pre[:, ft]
            wu_c = wu_pre[:, ft]
        else:
            wg_c = pfg.tile([P, DT, P], bf16, tag="wg", bufs=4, name=f"wg{ft}")
            nc.sync.dma_start(wg_c, wgT_d.ap()[:, ft])
            wu_c = pfg.tile([P, DT, P], bf16, tag="wu", bufs=4, name=f"wu{ft}")
            nc.gpsimd.dma_start(wu_c, wuT_d.ap()[:, ft])
        if ft % 4 == 2:
            # down-proj weights stream as 1MB chunks alternating across both
            # rings, interleaved with the g/u chunk stream
            i = ft // 4
            ring = nc.sync if i % 2 == 0 else nc.gpsimd
            ring.dma_start(wd_sb[:, 4 * i:4 * i + 4, :],
                           wdT_d.ap()[:, 4 * i:4 * i + 4, :])

        pg = psum.tile([P, 2, L], f32, tag="PR", bufs=2, name=f"pg{ft}")
        for dt in range(DT):
            nc.tensor.matmul(pg[:, 0, :], wg_c[:, dt, :], h2T[:, dt, :],
                             start=(dt == 0), stop=(dt == DT - 1))
        for dt in range(DT):
            nc.tensor.matmul(pg[:, 1, :], wu_c[:, dt, :], h2T[:, dt, :],
                             start=(dt == 0), stop=(dt == DT - 1))
        sl = pfg.tile([P, L], f32, tag="sl", bufs=2, name=f"sl{ft}")
        if _DBG:
            # CoreSim lacks Silu; emulate with sigmoid+mul for debugging
            nc.scalar.activation(sl, pg[:, 0, :],
                                 mybir.ActivationFunctionType.Sigmoid)
            sl2 = pfg.tile([P, L], f32, tag="sl2", bufs=2, name=f"sl2{ft}")
            nc.vector.tensor_mul(sl2, sl, pg[:, 0, :])
            sl = sl2
        else:
            nc.scalar.activation(sl, pg[:, 0, :], SILU)
        nc.vector.tensor_mul(fT[:, ft, :], sl, pg[:, 1, :])

    # down proj + residual + store (out carries C1; host divides); the final
    # chunk's adds alternate DVE/GpSimd and its stores use sync+scalar so no
    # single queue serializes the drain
    out_r = out_d.ap().rearrange("(lt p) d -> p lt d", p=P)
    rings = [nc.sync, nc.gpsimd, nc.scalar]
    for lt in range(LT):
        o_lt = pfg.tile([P, D], f32, tag="out", bufs=2, name=f"out{lt}")
        for dc in range(D // 512):
            # the very last chunk runs as two 256-col psum groups so its
            # add+store drain overlaps the second group's matmuls
            last = (lt == LT - 1 and dc == 1)
            ngr = 2 if last else 1
            wgr = 512 // ngr
            for gr in range(ngr):
                g0 = dc * 512 + gr * wgr
                psdn = psum.tile([P, wgr], f32, tag="CX", bufs=2,
                                 name=f"psdn{lt}_{dc}_{gr}")
                for ft in range(FTL):
                    nc.tensor.matmul(
                        psdn, fT[:, ft, lt * P:(lt + 1) * P],
                        wd_sb[:, ft, g0:g0 + wgr],
                        start=(ft == 0), stop=(ft == FTL - 1))
                nhf = 2
                wd_ = wgr // nhf
                for hf in range(nhf):
                    sl0 = g0 + hf * wd_
                    nc.vector.tensor_tensor(
                        o_lt[:, sl0:sl0 + wd_],
                        psdn[:, hf * wd_:(hf + 1) * wd_],
                        x2_sb[:, lt, sl0:sl0 + wd_], ADD)
                    ring = ([nc.sync, nc.scalar][(2 * gr + hf) % 2] if last
                            else rings[(2 * lt + dc * nhf + hf) % 3])
                    ring.dma_start(out_r[:, lt, sl0:sl0 + wd_],
                                   o_lt[:, sl0:sl0 + wd_])
    pfg.release()
    psum.release()
    persistH.release()
    persist2.release()
    consts.release()


def _to_bf16(a):
    return np.ascontiguousarray(a.astype(ml_dtypes.bfloat16))


def _to_f8(a, scale):
    y = np.asarray(a, np.float32) * np.float32(scale)
    np.clip(y, -240.0, 240.0, out=y)
    return np.ascontiguousarray(y.astype(ml_dtypes.float8_e4m3fn))


def prepare_core_inputs(x, text_k, text_v, ln1_w, ln2_w, Wq, Wk, Wv, Wo, Wg, Wu, Wd):
    """Host-side preprocessing: transpose weights, fold RMSNorm gammas,
    quantize (fp8 for Wq/Wk/Wv/Wo, bf16 elsewhere), prescale x by C1, and
    pre-arrange every tensor to the device [P, chunk, ...] layout so DMAs
    move KB-contiguous per-partition elements."""
    x = np.asarray(x, np.float32)

    def arr_pmaj(a):
        # [N*P, M] -> [P, N, M] (N chunk-major per partition)
        n = a.shape[0] // P
        return np.ascontiguousarray(
            a.reshape(n, P, a.shape[1]).transpose(1, 0, 2))

    def arr_kv(a):
        # [E, LC] -> [P, cc(4), ft(2), 512]: chunk-major contiguous per part
        return np.ascontiguousarray(
            a.reshape(2, P, 4, 512).transpose(1, 2, 0, 3))

    wq = _to_f8((np.asarray(Wq) * np.asarray(ln1_w)[None, :]).T, S_WQ)  # [D, O]
    # [D, O] -> [P, DT, O] -> chunks of 512 o-cols, hp-major: [P, 4, DT, 512]
    wq = wq.reshape(DT, P, 4, 512).transpose(1, 2, 0, 3)
    wo = _to_f8(np.asarray(Wo).T, S_WO)                 # [O, D]
    wo = wo.reshape(OT // 2, 2, P, 2, 512).transpose(2, 0, 3, 1, 4)
    wg = _to_bf16((np.asarray(Wg) * np.asarray(ln2_w)[None, :]).T)  # [D, F]
    wg = wg.reshape(DT, P, FTL, P).transpose(1, 2, 0, 3)            # [P,FTL,DT,P]
    # Wu carries C1 so the down-proj PSUM matches x2_sb's scale in the
    # final residual add (host divides the output by C1)
    wu = _to_bf16((np.asarray(Wu) * np.asarray(ln2_w)[None, :]).T
                  * np.float32(C1))
    wu = wu.reshape(DT, P, FTL, P).transpose(1, 2, 0, 3)
    shared = {
        "wqT": np.ascontiguousarray(wq),
        "wkT": arr_pmaj(_to_f8(np.asarray(Wk).T, S_WK)),
        "wvT": arr_pmaj(_to_f8(np.asarray(Wv).T, S_WK)),
        "woT": np.ascontiguousarray(wo),
        "wgT": np.ascontiguousarray(wg),
        "wuT": np.ascontiguousarray(wu),
        "wdT": arr_pmaj(_to_bf16(np.asarray(Wd).T)),   # [P, FTL, D]
    }
    in_maps = []
    for b in range(B):
        in_maps.append({
            "x": arr_pmaj(_to_bf16(np.asarray(x[b], np.float32)
                                   * np.float32(C1))),
            "tkT": arr_kv(_to_f8(np.asarray(text_k[b]).T, 16.0)),
            "tvT": arr_kv(_to_f8(np.asarray(text_v[b]).T, 16.0)),
            **shared,
        })
    return in_maps


_NC_CACHE = {}


def kernel(**inputs):
    if "nc" not in _NC_CACHE:
        _NC_CACHE["nc"] = build_program()
    nc = _NC_CACHE["nc"]
    in_maps = prepare_core_inputs(**inputs)
    res = run_bass_kernel_spmd(nc, in_maps, core_ids=list(range(B)))
    inv = np.float32(1.0 / C1)
    return np.stack([r["out"] * inv for r in res.results], axis=0)


if __name__ == "__main__":
    # smoke build
    nc = build_program()
    print("program built ok")


# revision 60
# speedup vs baseline: 1.0105x; 1.0064x over previous
"""Trainium2 Bass kernel for ActionExpertCrossBlock (dense transformer block
with GQA cross-attention + SwiGLU FFN), data-parallel over batch on 8 cores.

Contract: kernel(**inputs) takes the FULL fp32 inputs as produced by
setup_inputs() and returns the FULL [8, 512, 1024] fp32 output.

Per-core computation (batch element b):
  h   = rmsnorm(x) * ln1_w
  q   = rope((h @ Wq.T).reshape(L, 8, 256))
  k   = text_k @ Wk.T          (single KV head, shared by all 8 Q heads)
  v   = text_v @ Wv.T
  s_h = q_h @ k.T / 16         -> softmax over context
  ctx = attn @ v ; x2 = ctx @ Wo.T + x
  h2  = rmsnorm(x2) * ln2_w
  out = (silu(h2@Wg.T) * (h2@Wu.T)) @ Wd.T + x2

Precision: attention block (k/v-proj, q-proj, scores, softmax, attn@v,
o-proj) in fp8-e4m3 with DoubleRow matmuls (2 fp8 MACs per PE cell/cycle);
FFN stays bf16 (fp8 there blows the absmax error budget).  All quantization
scales are powers of two folded into existing constants.  Scores are computed
TRANSPOSED ([c, l]) so attn@v needs no on-chip transpose; the softmax
denominator uses a DoubleRow ones-matmul; exp = exp(s/16 - 3.5) stays < 240
(TRN e4m3 overflow).

Schedule (v2): the q-projection + RoPE of head h+1 are interleaved INTO head
h's attention stream (PE: 2 qproj half-groups; DVE: psum->bf16 copies + rope)
so there is no standalone q-proj phase with ACT idle; exp activations are
emitted 1024-wide (one per score PAIR, psum tile [P,2,L] spanning 2 banks) to
halve ACT instruction overhead -- ACT (8 exps/head ~9.5us) then just undercuts
PE (~10.3us/head).  The softmax reciprocal uses the ~5x faster
reciprocal_approx_fast (18 bits >> fp8 needs).  All bulk weight DMAs ride the
sync/gpsimd HWDGE rings so the Scalar(ACT) and Vector(DVE) queues carry no
descriptor work mid-kernel.  o-proj opens lt0/lt1 psum groups with heads 0-6
while head 7's recip/ctxT drain, then finishes with head 7.  PSUM map
(8 banks): PR 2x[P,2,L] (scores pairs / o-proj lt-groups / FFN gate+up
pairs), CX 2x[P,L] (k-proj / attn@v / norm2 transposes / down-proj), QP
1x[P,L] (qproj halves / norm1 transposes), DN 1x[P,L] (softmax denominator /
norm1 transposes).
"""
import sys

sys.path.insert(0, "/opt/trn_rl_repo")

import numpy as np
import ml_dtypes

import concourse.bass as bass
from concourse import bacc
import concourse.mybir as mybir
import concourse.tile as tile
from concourse.masks import make_identity
from concourse.bass_utils import run_bass_kernel_spmd

import os as _os
_DBG = _os.environ.get("ANT_DBG", "0") == "1"

P = 128
B, L, D = 8, 512, 1024
QH, HD = 8, 256
E = 256        # kv dim (1 head x 256)
LC = 2048      # context length
F = 4096       # ffn dim
O = QH * HD    # 2048
LT, DT, OT, CT, FTL = L // P, D // P, O // P, LC // P, F // P  # 4 8 16 16 32
f32, bf16, f8 = mybir.dt.float32, mybir.dt.bfloat16, mybir.dt.float8e4
DR = mybir.MatmulPerfMode.DoubleRow
EPS = float(np.finfo(np.float32).eps)
EXPF = mybir.ActivationFunctionType.Exp
SILU = mybir.ActivationFunctionType.Silu
SQRT = mybir.ActivationFunctionType.Sqrt
COPY = mybir.ActivationFunctionType.Copy
MUL = mybir.AluOpType.mult
SUB = mybir.AluOpType.subtract
ADD = mybir.AluOpType.add

# fp8 scale schedule (all powers of 2; see module docstring)
S_H = 16.0      # h = s_h * rmsnorm(x), fp8
S_WQ = 256.0    # Wq host-quant scale
S_Q = 16.0      # rope(q) fp8 scale; rope tables carry s_q/(s_h*s_wq) = 2^-8
S_K = 16.0      # k fp8 scale (copy from kv-proj psum)
S_V = 16.0      # v fp8 scale
S_WK = 32.0     # Wk/Wv host-quant scale (kv-proj runs fp8 DR)
S_CTX = 8.0     # ctx fp8 scale; ones-matrix = s_v/s_ctx = 2 folds it in
S_WO = 512.0    # Wo host-quant scale
C1 = S_CTX * S_WO          # 4096: x host-prescale == device x2/out scale
EXP_SCALE = 1.0 / (16.0 * S_Q * S_K)   # 2^-12
EXP_BIAS = -3.5
NPRE = 12       # wg/wu chunks prefetched during attention


def _rope_tables():
    # Match reference _rope numerics (fp32 ops) for d=256, l=512; tables are
    # pre-multiplied by s_q/(s_h*s_wq) so the DVE rope muls emit s_q*rope(q).
    d2 = HD // 2
    ts = (10000.0 ** (2.0 / HD * np.arange(d2, dtype=np.float32))).astype(np.float32)
    rad = (np.arange(L, dtype=np.float32)[None, :] / ts[:, None]).astype(np.float32)
    rs = np.float32(S_Q / (S_H * S_WQ))
    return (np.cos(rad) * rs).astype(ml_dtypes.bfloat16), \
        (np.sin(rad) * rs).astype(ml_dtypes.bfloat16)


def build_program():
    # All inputs are host-pre-arranged to [P(partition), chunk, ...] layouts so
    # every DMA moves KB-sized contiguous per-partition elements (no gathers).
    nc = bacc.Bacc()
    x_d = nc.dram_tensor("x", [P, LT, D], bf16, kind="ExternalInput")  # C1-scaled
    tkT_d = nc.dram_tensor("tkT", [P, 4, 2, 512], f8, kind="ExternalInput")  # x16
    tvT_d = nc.dram_tensor("tvT", [P, 4, 2, 512], f8, kind="ExternalInput")  # x16
    wqT_d = nc.dram_tensor("wqT", [P, 4, DT, 512], f8, kind="ExternalInput")
    wkT_d = nc.dram_tensor("wkT", [P, 2, E], f8, kind="ExternalInput")   # x32
    wvT_d = nc.dram_tensor("wvT", [P, 2, E], f8, kind="ExternalInput")   # x32
    woT_d = nc.dram_tensor("woT", [P, OT // 2, 2, 2, 512], f8, kind="ExternalInput")
    wgT_d = nc.dram_tensor("wgT", [P, FTL, DT, P], bf16, kind="ExternalInput")
    wuT_d = nc.dram_tensor("wuT", [P, FTL, DT, P], bf16, kind="ExternalInput")
    wdT_d = nc.dram_tensor("wdT", [P, FTL, D], bf16, kind="ExternalInput")
    out_d = nc.dram_tensor("out", [L, D], f32, kind="ExternalOutput")  # C1 * out

    cos_np, sin_np = _rope_tables()
    cosT_d = nc.inline_tensor(cos_np, "cosT")
    sinT_d = nc.inline_tensor(sin_np, "sinT")

    with tile.TileContext(nc) as tc:
        build_tile_kernel(
            tc, x_d, tkT_d, tvT_d, wqT_d, wkT_d, wvT_d, woT_d, wgT_d, wuT_d,
            wdT_d, cosT_d, sinT_d, out_d,
        )
    nc.compile()
    return nc


def _rmsnorm_lt(nc, pool, x_lt, dst_lt, sq_scale, eps_sb, tag, lt):
    """dst_lt = x_lt * rsqrt(ssum*sq_scale + eps_bias); sum-of-squares on ACT.

    sq_scale/eps_sb fold the h quantization scale and the host x prescale:
    dst = (s_out / (C * sqrt(mean((x/C)^2) + eps))) * x  for x = C*x_real,
    with sq_scale = 1/(D*s_out^2) and eps_bias = (C/s_out)^2 * eps.
    """
    sq = pool.tile([P, D], f32, tag=f"{tag}_sq", bufs=2, name=f"{tag}sq{lt}")
    ssum = pool.tile([P, 1], f32, tag=f"{tag}_sum", bufs=2, name=f"{tag}sm{lt}")
    nc.scalar.activation(sq, x_lt, mybir.ActivationFunctionType.Square,
                         accum_out=ssum)
    std = pool.tile([P, 1], f32, tag=f"{tag}_std", bufs=2, name=f"{tag}sd{lt}")
    nc.scalar.activation(std, ssum, SQRT, scale=sq_scale, bias=eps_sb)
    rstd = pool.tile([P, 1], f32, tag=f"{tag}_rstd", bufs=2, name=f"{tag}rs{lt}")
    nc.vector.reciprocal(rstd, std)
    nc.vector.tensor_scalar_mul(dst_lt[:, :D // 2], x_lt[:, :D // 2], rstd)
    nc.vector.tensor_scalar_mul(dst_lt[:, D // 2:], x_lt[:, D // 2:], rstd)


def build_tile_kernel(tc, x_d, tkT_d, tvT_d, wqT_d, wkT_d, wvT_d, woT_d,
                      wgT_d, wuT_d, wdT_d, cosT_d, sinT_d, out_d):
    nc = tc.nc

    consts = tc.alloc_tile_pool(name="consts", bufs=1)
    persist2 = tc.alloc_tile_pool(name="persist2", bufs=1)  # x2: lives to the end
    persistH = tc.alloc_tile_pool(name="persistH", bufs=1)  # h2T: E..G
    persist1 = tc.alloc_tile_pool(name="persist1", bufs=1)  # dead after o-proj

    ident = consts.tile([P, P], bf16)
    ones8 = consts.tile([P, 2, P], f8)   # value 2 = s_v/s_ctx folded into den
    eps1_sb = consts.tile([P, 1], f32)   # (C1/S_H)^2 * eps
    eps2_sb = consts.tile([P, 1], f32)   # C1^2 * eps (h2 unscaled)
    ebias_sb = consts.tile([P, 1], f32)  # exp bias (cancels in softmax)
    cosT = consts.tile([P, L], bf16)
    sinT = consts.tile([P, L], bf16)

    # ---- persistent activations (split into per-slice tiles so consumers
    # depend only on the slices they read, not on whole-tensor last-writes) ----
    qT_t = [persist1.tile([P, 2, L], f8, tag="qT", bufs=QH, name=f"qT{h}")
            for h in range(QH)]
    kT = persist1.tile([P, 2, LC], f8)
    v_sb = persist1.tile([P, CT, E], f8)
    ctxT_t = [persist1.tile([P, 2, L], f8, tag="ctxT", bufs=QH, name=f"ctxT{h}")
              for h in range(QH)]
    x_sb = persist1.tile([P, LT, D], bf16)
    x_t = [x_sb[:, lt, :] for lt in range(LT)]
    hT = persist1.tile([P, DT, L], f8)   # norm1(x) transposed, fp8 (S_H)
    x2_sb = persist2.tile([P, LT, D], f32)

    patt = tc.alloc_tile_pool(name="patt", bufs=1)   # attention-phase tiles
    pexp = tc.alloc_tile_pool(name="pexp", bufs=1)   # attention exp/recip tiles
    psum = tc.alloc_tile_pool(name="psum", bufs=2, space="PSUM")

    # ======== input DMAs: tiny/early loads spread over all HWDGE rings
    # (scalar only carries startup loads; mid-kernel descriptor work stays
    # on sync+gpsimd so the ACT queue is compute-only) ========
    wk_sb = patt.tile([P, 2, E], f8)
    nc.scalar.dma_start(wk_sb, wkT_d.ap())
    wv_sb = patt.tile([P, 2, E], f8)
    nc.scalar.dma_start(wv_sb, wvT_d.ap())

    # tk chunks split across sync/scalar in CONSUMPTION halves (tk0/tk1 on
    # sync, tk2/tk3 on scalar) so they land in the order k-proj consumes them
    tk_c = []
    for cc in range(4):
        t = patt.tile([P, 2, 512], f8, tag="tk", bufs=4, name=f"tk{cc}")
        (nc.sync if cc < 2 else nc.scalar).dma_start(
            t, tkT_d.ap()[:, cc, :, :])
        tk_c.append(t)

    # x chunks split across sync+scalar so neither ring serializes the 1MB
    # behind the tk stream (norm1 lt=3 gates the whole q-proj/rope chain)
    for lt in range(LT):
        (nc.sync if lt % 2 == 0 else nc.scalar).dma_start(
            x_sb[:, lt, :], x_d.ap()[:, lt, :])

    tv_c = []
    for cc in range(4):
        t = patt.tile([P, 2, 512], f8, tag="tv", bufs=4, name=f"tv{cc}")
        nc.gpsimd.dma_start(t, tvT_d.ap()[:, cc, :, :])
        tv_c.append(t)
    nc.gpsimd.dma_start(cosT, cosT_d.ap())
    nc.gpsimd.dma_start(sinT, sinT_d.ap())
    wq_sb = patt.tile([P, 4, DT, 512], f8)
    # identity after the startup-critical gpsimd DMAs (needed ~13us)
    make_identity(nc, ident)
    wq_c = [wq_sb[:, h // 2, :, (h % 2) * 256:(h % 2) * 256 + 256]
            for h in range(QH)]

    nc.vector.memset(ones8, 2.0)
    nc.vector.memset(eps1_sb, (C1 / S_H) ** 2 * EPS)
    nc.vector.memset(eps2_sb, C1 * C1 * EPS)
    nc.vector.memset(ebias_sb, EXP_BIAS)

    # ====== k/v projections (fp8 DoubleRow) interleaved with norm1 and the
    # hT transposes -- the transposes fill PE while later chunks stream ======
    KSC = S_K / (16.0 * S_WK)   # psum = (16*tk)(32*wk) = 512*k
    VSC = S_V / (16.0 * S_WK)
    h_lts = []

    def emit_norm1(lt):
        h_lt = patt.tile([P, D], bf16, tag="h_bf", bufs=4, name=f"hbf{lt}")
        _rmsnorm_lt(nc, patt, x_t[lt], h_lt, 1.0 / (D * S_H * S_H),
                    eps1_sb, "n1", lt)
        h_lts.append(h_lt)

    def emit_trans1(lt, wide=False):
        # 4-way psum rotation once the kv matmuls (CX/PR users) are emitted
        tags = ("QP", "DN", "CX", "PR") if wide else ("QP", "DN")
        bufs = {"QP": 1, "DN": 1, "CX": 2, "PR": 2}
        for dt in range(DT):
            tag = tags[dt % len(tags)]
            tp = psum.tile([P, P], bf16, tag=tag, bufs=bufs[tag],
                           name=f"atp{lt}_{dt}")
            nc.tensor.transpose(tp, h_lts[lt][:, dt * P:(dt + 1) * P], ident)
            if dt % 2 == 0:
                nc.scalar.activation(hT[:, dt, lt * P:(lt + 1) * P], tp, COPY)
            else:
                nc.vector.tensor_copy(hT[:, dt, lt * P:(lt + 1) * P], tp)

    for cc in range(4):
        c0 = cc * 512
        for et in range(2):
            psk = psum.tile([P, 512], f32, tag="CX", bufs=2,
                            name=f"psk{et}_{cc}")
            nc.tensor.matmul(psk, wk_sb[:, :, et * P:(et + 1) * P],
                             tk_c[cc], start=True, stop=True, perf_mode=DR)
            if (cc + et) % 2 == 0:
                nc.vector.tensor_scalar_mul(kT[:, et, c0:c0 + 512], psk, KSC)
            else:
                nc.scalar.activation(kT[:, et, c0:c0 + 512], psk, COPY,
                                     scale=KSC)
        for ct in range(4 * cc, 4 * cc + 4):
            psv = psum.tile([P, E], f32, tag="PR", bufs=2, name=f"psv{ct}")
            nc.tensor.matmul(
                psv, tv_c[cc][:, :, (ct % 4) * P:(ct % 4 + 1) * P],
                wv_sb, start=True, stop=True, perf_mode=DR)
            if ct % 2 == 0:
                nc.scalar.activation(v_sb[:, ct, :], psv, COPY, scale=VSC)
            else:
                nc.vector.tensor_scalar_mul(v_sb[:, ct, :], psv, VSC)
        emit_norm1(cc)
        if cc == 0:
            # preload the ACT exp table during prefill (a table switch costs
            # ~1.3us; don't pay it at head 0's first exp)
            dmex = patt.tile([P, 1], f8, name="dmex")
            nc.scalar.activation(dmex, eps1_sb, EXPF, scale=1.0, bias=ebias_sb)
            # wq chunks 1-3 (heads 2-7, not needed before ~55us) are gated
            # behind norm1(0)'s output so they stay out of the startup HBM
            # burst that the tk/tv/x chunk loads depend on
            nc.gpsimd.dma_start(wq_sb[:, 0], wqT_d.ap()[:, 0])
            for hp in range(1, 4):
                nc.vector.tensor_scalar_mul(wq_sb[0:1, hp, 0, 0:2],
                                            h_lts[0][0:1, 0:2], 0.0)
                nc.gpsimd.dma_start(wq_sb[:, hp], wqT_d.ap()[:, hp])
        if cc >= 1:
            emit_trans1(cc - 1, wide=(cc == 3))
    emit_trans1(LT - 1, wide=True)

    # ---- per-head q-proj + rope helpers (emitted inside head h-1's stream) --
    def qproj_half(g, half):
        psq = psum.tile([P, L], f32, tag="QP", bufs=1, name=f"psq{g}_{half}")
        for i in range(DT // 2):
            nc.tensor.matmul(
                psq, wq_c[g][:, 2 * i:2 * i + 2, half * P:(half + 1) * P],
                hT[:, 2 * i:2 * i + 2, :],
                start=(i == 0), stop=(i == DT // 2 - 1), perf_mode=DR)
        return psq

    def pqcopy(g, half, psq):
        t = patt.tile([P, L], bf16, tag="pqb", bufs=4, name=f"pqb{g}_{half}")
        nc.vector.tensor_copy(t, psq)
        return t

    def rope(g, pq0, pq1):
        # x1 = pq0, x2 = pq1 ([hd_j, l] layout; tables [j, l] carry the
        # s_q/(s_h*s_wq) rescale); outputs quantize to fp8.
        t_a = patt.tile([P, L], bf16, tag="rope_t", bufs=4, name=f"ta{g}")
        nc.vector.tensor_mul(t_a, pq0, cosT)
        t_b = patt.tile([P, L], bf16, tag="rope_t", bufs=4, name=f"tb{g}")
        nc.vector.tensor_mul(t_b, pq1, sinT)
        nc.vector.tensor_tensor(qT_t[g][:, 0, :], t_a, t_b, SUB)
        t_c = patt.tile([P, L], bf16, tag="rope_t", bufs=4, name=f"tc{g}")
        nc.vector.tensor_mul(t_c, pq1, cosT)
        t_d = patt.tile([P, L], bf16, tag="rope_t", bufs=4, name=f"td{g}")
        nc.vector.tensor_mul(t_d, pq0, sinT)
        nc.vector.tensor_tensor(qT_t[g][:, 1, :], t_c, t_d, ADD)

    # prefill: head 0's q-proj + rope, then head 1's first q-proj half (the
    # per-head loop emits half0 of head g at the END of head g-2's stream so
    # its psum copy lands on DVE before the boundary recip chain)
    psq0 = qproj_half(0, 0)
    pq0 = pqcopy(0, 0, psq0)
    psq1 = qproj_half(0, 1)
    pq1 = pqcopy(0, 1, psq1)
    rope(0, pq0, pq1)
    nxt = {}
    nxt[1] = qproj_half(1, 0)
    nxt[(1, "pq")] = pqcopy(1, 0, nxt[1])

    # ---- o-proj weight layout + FFN weight prefetch (issued once DVE reaches
    # the gate memsets, i.e. after head-0's rope -- keeps startup HBM clear) --
    pde = tc.alloc_tile_pool(name="ph_de", bufs=1)
    # wo layout [p, ot-pair, dc, parity, col]: each (pair, dc) moving slice is
    # a contiguous 1024B row pair -- strided DR moving operands run half-rate
    wo_sb = pde.tile([P, OT // 2, 2, 2, 512], f8)   # 16KB/part
    wg_pre = persistH.tile([P, NPRE, DT, P], bf16)
    wu_pre = persistH.tile([P, NPRE, DT, P], bf16)
    # WAW-gate the bulk prefetches on dummy writes that READ qT0 (produced by
    # head-0's rope): a bare memset has no deps and gets hoisted to t=0 by the
    # scheduler, and the 8MB prefetch then starves the startup loads of HBM
    nc.vector.tensor_scalar_mul(wo_sb[0:1, 0, 0, 0, 0:2], qT_t[0][0:1, 0, 0:2],
                                0.0)
    nc.vector.tensor_scalar_mul(wg_pre[0:1, 0, 0, 0:2], qT_t[0][0:1, 0, 0:2],
                                0.0)
    nc.vector.tensor_scalar_mul(wu_pre[0:1, 0, 0, 0:2], qT_t[0][0:1, 0, 0:2],
                                0.0)
    nc.sync.dma_start(wo_sb, woT_d.ap())
    nc.sync.dma_start(wg_pre, wgT_d.ap()[:, :NPRE])
    nc.gpsimd.dma_start(wu_pre, wuT_d.ap()[:, :NPRE])

    # ---- o-proj accumulator (dc-granular so head 7 can prefill lt0/dc0
    # into the QP bank while its own attention drains) ----
    pso_open = {}

    def oproj_acc(lt, dc, i0, i1, tag):
        if (lt, dc) not in pso_open:
            if tag == "PR":
                t = psum.tile([P, 2, 512], f32, tag="PR", bufs=2,
                              name=f"pso{lt}")
                pso_open[(lt, 0)] = t[:, 0, :]
                pso_open[(lt, 1)] = t[:, 1, :]
            else:
                pso_open[(lt, dc)] = psum.tile([P, 512], f32, tag=tag, bufs=1,
                                               name=f"pso{lt}_{dc}")
        pso = pso_open[(lt, dc)]
        for i in range(i0, i1):
            nc.tensor.matmul(
                pso, ctxT_t[i][:, :, lt * P:(lt + 1) * P],
                wo_sb[:, i, dc, :, :],
                start=(i == 0), stop=(i == OT // 2 - 1), perf_mode=DR)
        if i1 == OT // 2:
            nc.vector.tensor_tensor(
                x2_sb[:, lt, dc * 512:(dc + 1) * 512], pso,
                x_t[lt][:, dc * 512:(dc + 1) * 512], ADD)
            del pso_open[(lt, dc)]

    # ============ attention: per-head fused loop ============
    for h in range(QH):
        g = h + 1
        psc = [psum.tile([P, L], f32, tag="CX", bufs=2, name=f"psc{h}_{et}")
               for et in range(2)]
        psd = psum.tile([P, L], f32, tag="DN", bufs=1, name=f"psd{h}")
        exps = [None] * (CT // 2)

        def sp(p, h=h, exps=exps):
            # score pair p (ct = 2p, 2p+1) -> one 1024-wide fp8 exp on ACT
            pr = psum.tile([P, 2, L], f32, tag="PR", bufs=2, name=f"pr{h}_{p}")
            for j in range(2):
                ct = 2 * p + j
                nc.tensor.matmul(pr[:, j, :], kT[:, :, ct * P:(ct + 1) * P],
                                 qT_t[h], start=True, stop=True, perf_mode=DR)
            ex = pexp.tile([P, 2, L], f8, tag="exp", bufs=8, name=f"ex{h}_{p}")
            nc.scalar.activation(ex, pr, EXPF, scale=EXP_SCALE, bias=ebias_sb)
            exps[p] = ex

        def ctx(i, psc=psc, exps=exps):
            for et in range(2):
                nc.tensor.matmul(
                    psc[et], v_sb[:, 2 * i:2 * i + 2, et * P:(et + 1) * P],
                    exps[i], start=(i == 0), stop=(i == CT // 2 - 1),
                    perf_mode=DR)

        def den(i, psd=psd, exps=exps):
            nc.tensor.matmul(psd, ones8, exps[i],
                             start=(i == 0), stop=(i == CT // 2 - 1),
                             perf_mode=DR)

        # software pipeline: ctx(0) is deferred so its psum WAR (prev head's
        # ctxT muls draining on DVE) clears before PE arrives; head g's
        # q-proj half0 was emitted at the end of head h-1 (copy on DVE before
        # the boundary recip), half1+rope ride in this head's early slots
        sp(0)
        sp(1)
        if g < QH:
            psq1 = qproj_half(g, 1)
        sp(2)
        if g < QH:
            pq1 = pqcopy(g, 1, psq1)
        ctx(0)
        sp(3)
        if g < QH:
            rope(g, nxt[(g, "pq")], pq1)
        ctx(1)
        den(0)
        for p in range(4, CT // 2):
            sp(p)
            ctx(p - 2)
            den(p - 3)
        ctx(CT // 2 - 2)
        den(CT // 2 - 3)
        ctx(CT // 2 - 1)
        den(CT // 2 - 2)
        den(CT // 2 - 1)
        if g + 1 < QH:
            # next-next head's q-proj half0 + its DVE copy, ahead of the
            # boundary reciprocal so PE's half1 never waits on the copy
            nxt[g + 1] = qproj_half(g + 1, 0)
            nxt[(g + 1, "pq")] = pqcopy(g + 1, 0, nxt[g + 1])

        if _DBG and h == 0:
            dbg_ex = nc.dram_tensor("dbg_ex0", [P, CT // 2, 2, L], f8,
                                    kind="ExternalOutput")
            for p in range(CT // 2):
                nc.sync.dma_start(dbg_ex.ap()[:, p, :, :], exps[p])
        recip = pexp.tile([P, L], f32, tag="recip", bufs=2, name=f"rc{h}")
        nc.vector.reciprocal_approx_fast(recip, psd)
        for et in range(2):
            nc.vector.tensor_mul(ctxT_t[h][:, et, :], psc[et], recip)
        if _DBG:
            nc.sync.dma_start(
                nc.dram_tensor(f"dbg_rc{h}", [P, L], f32,
                               kind="ExternalOutput").ap(), recip)

    # ------- o-proj (fp8 DR) + residual, interleaved with norm2 ---------
    # lt0/dc0 was prefilled (heads 0..5) inside head 7's stream; lt1/lt2 use
    # the PR pair tiles, lt0/lt3 the QP+DN single banks
    h2T = persistH.tile([P, DT, L], bf16)

    def emit_norm2_rms(lt, h2_lt):
        _rmsnorm_lt(nc, pde, x2_sb[:, lt, :], h2_lt, 1.0 / D, eps2_sb,
                    "n2", lt)

    def emit_norm2_T(lt, h2_lt):
        for dt in range(DT):
            tp = psum.tile([P, P], bf16, tag="CX", bufs=2, name=f"ftp{lt}_{dt}")
            nc.tensor.transpose(tp, h2_lt[:, dt * P:(dt + 1) * P], ident)
            if dt % 2 == 0:
                nc.scalar.activation(h2T[:, dt, lt * P:(lt + 1) * P], tp, COPY)
            else:
                nc.vector.tensor_copy(h2T[:, dt, lt * P:(lt + 1) * P], tp)

    h2_t = [pde.tile([P, D], bf16, tag="h2bf", bufs=4, name=f"h2bf{lt}")
            for lt in range(LT)]
    # PR groups first: their psum slots have no dependence on head 7's
    # recip/ctxT drain (the QP/DN slots WAR-wait the psd read), and by the
    # time the head-7 stop-matmuls arrive ctxT7 is long ready
    QD = ("QP", "DN")
    for dc in range(2):
        oproj_acc(1, dc, 0, OT // 2, "PR")
    emit_norm2_rms(1, h2_t[1])
    for dc in range(2):
        oproj_acc(2, dc, 0, OT // 2, "PR")
    emit_norm2_T(1, h2_t[1])
    emit_norm2_rms(2, h2_t[2])
    for dc in range(2):
        oproj_acc(0, dc, 0, OT // 2, QD[dc])
    emit_norm2_T(2, h2_t[2])
    emit_norm2_rms(0, h2_t[0])
    for dc in range(2):
        oproj_acc(3, dc, 0, OT // 2, QD[dc])
    emit_norm2_T(0, h2_t[0])
    emit_norm2_rms(3, h2_t[3])
    emit_norm2_T(3, h2_t[3])

    if _DBG:
        nc.sync.dma_start(nc.dram_tensor("dbg_kT", [P, 2, LC], f8,
                                         kind="ExternalOutput").ap(), kT)
        nc.sync.dma_start(nc.dram_tensor("dbg_v", [P, CT, E], f8,
                                         kind="ExternalOutput").ap(), v_sb)
        nc.sync.dma_start(nc.dram_tensor("dbg_hT", [P, DT, L], f8,
                                         kind="ExternalOutput").ap(), hT)
        for h in range(QH):
            nc.sync.dma_start(nc.dram_tensor(f"dbg_qT{h}", [P, 2, L], f8,
                                             kind="ExternalOutput").ap(), qT_t[h])
            nc.sync.dma_start(nc.dram_tensor(f"dbg_ctxT{h}", [P, 2, L], f8,
                                             kind="ExternalOutput").ap(), ctxT_t[h])
        nc.sync.dma_start(nc.dram_tensor("dbg_x2", [P, LT, D], f32,
                                         kind="ExternalOutput").ap(), x2_sb)

    pde.release()
    pexp.release()
    patt.release()
    persist1.release()

    # ================= FFN (bf16) =================
    pfg = tc.alloc_tile_pool(name="ph_fg", bufs=1)
    fT = pfg.tile([P, FTL, L], bf16)          # 32KB/part

    wd_sb = pfg.tile([P, FTL, D], bf16)       # 64KB/part
    for ft in range(FTL):
        if ft < NPRE:
            wg_c = wg_pre[:, ft]
            wu_c = wu_pre[:, ft]
        else:
            wg_c = pfg.tile([P, DT, P], bf16, tag="wg", bufs=4, name=f"wg{ft}")
            nc.sync.dma_start(wg_c, wgT_d.ap()[:, ft])
            wu_c = pfg.tile([P, DT, P], bf16, tag="wu", bufs=4, name=f"wu{ft}")
            nc.gpsimd.dma_start(wu_c, wuT_d.ap()[:, ft])
        if ft % 4 == 2:
            # down-proj weights stream as 1MB chunks alternating across both
            # rings, interleaved with the g/u chunk stream
            i = ft // 4
            ring = nc.sync if i % 2 == 0 else nc.gpsimd
            ring.dma_start(wd_sb[:, 4 * i:4 * i + 4, :],
                           wdT_d.ap()[:, 4 * i:4 * i + 4, :])

        pg = psum.tile([P, 2, L], f32, tag="PR", bufs=2, name=f"pg{ft}")
        for dt in range(DT):
            nc.tensor.matmul(pg[:, 0, :], wg_c[:, dt, :], h2T[:, dt, :],
                             start=(dt == 0), stop=(dt == DT - 1))
        for dt in range(DT):
            nc.tensor.matmul(pg[:, 1, :], wu_c[:, dt, :], h2T[:, dt, :],
                             start=(dt == 0), stop=(dt == DT - 1))
        sl = pfg.tile([P, L], f32, tag="sl", bufs=2, name=f"sl{ft}")
        if _DBG:
            # CoreSim lacks Silu; emulate with sigmoid+mul for debugging
            nc.scalar.activation(sl, pg[:, 0, :],
                                 mybir.ActivationFunctionType.Sigmoid)
            sl2 = pfg.tile([P, L], f32, tag="sl2", bufs=2, name=f"sl2{ft}")
            nc.vector.tensor_mul(sl2, sl, pg[:, 0, :])
            sl = sl2
        else:
            nc.scalar.activation(sl, pg[:, 0, :], SILU)
        nc.vector.tensor_mul(fT[:, ft, :], sl, pg[:, 1, :])

    # down proj + residual + store (out carries C1; host divides); the final
    # chunk's adds alternate DVE/GpSimd and its stores use sync+scalar so no
    # single queue serializes the drain
    out_r = out_d.ap().rearrange("(lt p) d -> p lt d", p=P)
    rings = [nc.sync, nc.gpsimd, nc.scalar]
    for lt in range(LT):
        o_lt = pfg.tile([P, D], f32, tag="out", bufs=2, name=f"out{lt}")
        for dc in range(D // 512):
            # the very last chunk runs as two 256-col psum groups so its
            # add+store drain overlaps the second group's matmuls
            last = (lt == LT - 1 and dc == 1)
            ngr = 2 if last else 1
            wgr = 512 // ngr
            for gr in range(ngr):
                g0 = dc * 512 + gr * wgr
                psdn = psum.tile([P, wgr], f32, tag="CX", bufs=2,
                                 name=f"psdn{lt}_{dc}_{gr}")
                for ft in range(FTL):
                    nc.tensor.matmul(
                        psdn, fT[:, ft, lt * P:(lt + 1) * P],
                        wd_sb[:, ft, g0:g0 + wgr],
                        start=(ft == 0), stop=(ft == FTL - 1))
                nhf = 2
                wd_ = wgr // nhf
                for hf in range(nhf):
                    sl0 = g0 + hf * wd_
                    nc.vector.tensor_tensor(
                        o_lt[:, sl0:sl0 + wd_],
                        psdn[:, hf * wd_:(hf + 1) * wd_],
                        x2_sb[:, lt, sl0:sl0 + wd_], ADD)
                    ring = ([nc.sync, nc.scalar][(2 * gr + hf) % 2] if last
                            else rings[(2 * lt + dc * nhf + hf) % 3])
                    ring.dma_start(out_r[:, lt, sl0:sl0 + wd_],
                                   o_lt[:, sl0:sl0 + wd_])
    pfg.release()
    psum.release()
    persistH.release()
    persist2.release()
    consts.release()


def _to_bf16(a):
    return np.ascontiguousarray(a.astype(ml_dtypes.bfloat16))


def _to_f8(a, scale):
    y = np.asarray(a, np.float32) * np.float32(scale)
    np.clip(y, -240.0, 240.0, out=y)
    return np.ascontiguousarray(y.astype(ml_dtypes.float8_e4m3fn))


def prepare_core_inputs(x, text_k, text_v, ln1_w, ln2_w, Wq, Wk, Wv, Wo, Wg, Wu, Wd):
    """Host-side preprocessing: transpose weights, fold RMSNorm gammas,
    quantize (fp8 for Wq/Wk/Wv/Wo, bf16 elsewhere), prescale x by C1, and
    pre-arrange every tensor to the device [P, chunk, ...] layout so DMAs
    move KB-contiguous per-partition elements."""
    x = np.asarray(x, np.float32)

    def arr_pmaj(a):
        # [N*P, M] -> [P, N, M] (N chunk-major per partition)
        n = a.shape[0] // P
        return np.ascontiguousarray(
            a.reshape(n, P, a.shape[1]).transpose(1, 0, 2))

    def arr_kv(a):
        # [E, LC] -> [P, cc(4), ft(2), 512]: chunk-major contiguous per part
        return np.ascontiguousarray(
            a.reshape(2, P, 4, 512).transpose(1, 2, 0, 3))

    wq = _to_f8((np.asarray(Wq) * np.asarray(ln1_w)[None, :]).T, S_WQ)  # [D, O]
    # [D, O] -> [P, DT, O] -> chunks of 512 o-cols, hp-major: [P, 4, DT, 512]
    wq = wq.reshape(DT, P, 4, 512).transpose(1, 2, 0, 3)
    wo = _to_f8(np.asarray(Wo).T, S_WO)                 # [O, D]
    wo = wo.reshape(OT // 2, 2, P, 2, 512).transpose(2, 0, 3, 1, 4)
    wg = _to_bf16((np.asarray(Wg) * np.asarray(ln2_w)[None, :]).T)  # [D, F]
    wg = wg.reshape(DT, P, FTL, P).transpose(1, 2, 0, 3)            # [P,FTL,DT,P]
    # Wu carries C1 so the down-proj PSUM matches x2_sb's scale in the
    # final residual add (host divides the output by C1)
    wu = _to_bf16((np.asarray(Wu) * np.asarray(ln2_w)[None, :]).T
                  * np.float32(C1))
    wu = wu.reshape(DT, P, FTL, P).transpose(1, 2, 0, 3)
    shared = {
        "wqT": np.ascontiguousarray(wq),
        "wkT": arr_pmaj(_to_f8(np.asarray(Wk).T, S_WK)),
        "wvT": arr_pmaj(_to_f8(np.asarray(Wv).T, S_WK)),
        "woT": np.ascontiguousarray(wo),
        "wgT": np.ascontiguousarray(wg),
        "wuT": np.ascontiguousarray(wu),
        "wdT": arr_pmaj(_to_bf16(np.asarray(Wd).T)),   # [P, FTL, D]
    }
    in_maps = []
    for b in range(B):
        in_maps.append({
            "x": arr_pmaj(_to_bf16(np.asarray(x[b], np.float32)
                                   * np.float32(C1))),
            "tkT": arr_kv(_to_f8(np.asarray(text_k[b]).T, 16.0)),
            "tvT": arr_kv(_to_f8(np.asarray(text_v[b]).T, 16.0)),
            **shared,
        })
    return in_maps


_NC_CACHE = {}


def kernel(**inputs):
    if "nc" not in _NC_CACHE:
        _NC_CACHE["nc"] = build_program()
    nc = _NC_CACHE["nc"]
    in_maps = prepare_core_inputs(**inputs)
    res = run_bass_kernel_spmd(nc, in_maps, core_ids=list(range(B)))
    inv = np.float32(1.0 / C1)
    return np.stack([r["out"] * inv for r in res.results], axis=0)


if __name__ == "__main__":
    # smoke build
    nc = build_program()
    print("program built ok")
